# revision 15
# baseline (speedup 1.0000x reference)
"""Cross-attention (q-norm variant) Trainium2 Bass kernel, v4.

Sharding: batch (2) x row-quarters (4) -> 8 cores, data-parallel over the
query sequence. Each core handles 1408 query rows (5376 padded to 5632 per
batch) of ONE batch, with that batch's context replicated. kv projection is
computed 1/4-per-core within each batch group and all-gathered.

v4 changes over v2 (all scheduling / engine-balance; math identical):
  - DMA issue is sequencer-FIFO-ordered with ~0.6-1us issue cost and
    head-of-line sem blocking, so DMAs are laid out per-ring by priority:
    sync ring gets the kv-feeding loads (coarse chunks), then wq/wo/x
    prefetches, then kv_part export (whose sem wait parks the ring at a
    point where nothing else is pending); the collective gather-back DMAs
    live on the gpsimd SWDGE ring whose next work (rowsum adds) isn't
    needed until attention anyway.  The collective now dispatches at
    ~27us instead of ~64us.
  - softmax row sums: the 4 ones-matmuls per head are replaced by
    gpsimd/DVE partial sums over the 4 j-blocks (pT0+pT1, pT2+pT3 on
    gpsimd, final add on DVE) + ONE ones-matmul -> 3/4 of that PE time
    moves to otherwise-idle engines.
  - attention is software-pipelined one head deep: head h's scores+exp are
    emitted before head h-1's AV/rowsum/normalize, so the exp latency is
    covered by real PE work instead of stalling.
  - AV matmuls run before the rowsum matmul (rowsum partials are still in
    flight on gpsimd while AV streams).
  - prologue projects PRO=7 q row-blocks before attention group 0 so the
    PE stays busy through the kv all-gather (~35us payload transfer).
  - outproj PSUM->SBUF copies moved from ACT to DVE (ACT keeps only exp
    and q-proj copies).

Host-side prep (numpy): cast weights to bf16, fold q_norm_scale into the
k-half of wkv, transpose x and context. Biases are structurally zero in
this problem and are dropped.
"""

import sys
import numpy as np

for _p in ("/opt/trn_rl_repo",):
    if _p not in sys.path:
        sys.path.insert(0, _p)

import ml_dtypes
import concourse.bass as bass
import concourse.tile as tile
from concourse import bacc, mybir
from concourse import bass_utils
from concourse.masks import make_identity

F32 = mybir.dt.float32
BF16 = mybir.dt.bfloat16
I32 = mybir.dt.int32
EXP = mybir.ActivationFunctionType.Exp
MULT = mybir.AluOpType.mult
ADD = mybir.AluOpType.add
LSR = mybir.AluOpType.logical_shift_right
XOR = mybir.AluOpType.bitwise_xor

B, N, D, M, H, HD = 2, 5376, 1536, 512, 12, 128
EPS = 1e-6
EPSH = float(HD * EPS)
NCORES = 8
CPB = 4            # cores per batch
RPC = 1408         # padded rows per core  (4*1408 = 5632 >= 5376)
NBLK = RPC // 128  # 11
DC = D // 128      # 12 contraction chunks
JB = M // 128      # 4 context row blocks
GROUPS = [(0, 4), (4, 4), (8, 3)]   # (start block, #blocks)
PRO = 8            # q row-blocks projected before attention starts
MAGIC_P1 = 0x5F3759E0  # quake rsqrt magic + 1

TRACE = False

_cache = {}


def _build(reps=1):
    nc = bacc.Bacc(
        "TRN2", target_bir_lowering=False, debug=False, num_devices=NCORES
    )
    xT_d = nc.dram_tensor("xT", [D, RPC], BF16, kind="ExternalInput").ap()
    ctxT_d = nc.dram_tensor("ctxT", [D, M], BF16, kind="ExternalInput").ap()
    wq_d = nc.dram_tensor("wq", [D, D], BF16, kind="ExternalInput").ap()
    # per-core slices: 3 k-heads / 384 v-cols per core; kv is all-gathered
    wkp_d = nc.dram_tensor("wkp", [D, 384], BF16, kind="ExternalInput").ap()
    wvp_d = nc.dram_tensor("wvp", [D, 384], BF16, kind="ExternalInput").ap()
    wo_d = nc.dram_tensor("wo", [D, D], BF16, kind="ExternalInput").ap()
    out_d = nc.dram_tensor("out", [RPC, D], F32, kind="ExternalOutput").ap()

    xT_r = xT_d.rearrange("(c p) n -> p c n", p=128)      # [128, DC, RPC]
    ctxT_r = ctxT_d.rearrange("(c p) m -> p c m", p=128)  # [128, DC, M]
    wq_r = wq_d.rearrange("(c p) n -> p c n", p=128)
    wkp_r = wkp_d.rearrange("(c p) n -> p c n", p=128)
    wvp_r = wvp_d.rearrange("(c p) n -> p c n", p=128)
    wo_r = wo_d.rearrange("(c p) n -> p c n", p=128)

    with tile.TileContext(nc) as tc:
        with (
            tc.tile_pool(name="const", bufs=1) as constp,
            tc.tile_pool(name="wts", bufs=1) as wtp,
            tc.tile_pool(name="work", bufs=2) as workp,
            tc.tile_pool(name="dram", bufs=1, space="DRAM") as dramp,
            tc.tile_pool(name="ps", bufs=2, space="PSUM") as psp,
        ):
            ident_b = constp.tile([128, 128], BF16, name="ident_b")
            make_identity(nc, ident_b)
            ones_b = constp.tile([128, 128], BF16, name="ones_b")
            nc.vector.memset(ones_b[:], 1.0)

            wq_sb = wtp.tile([128, DC, D], BF16, name="wq_sb")
            wo_sb = wtp.tile([128, DC, D], BF16, name="wo_sb")
            kT_sb = wtp.tile([128, H, M], BF16, name="kT_sb")   # [d, h, j]
            v_sb = wtp.tile([128, JB, D], BF16, name="v_sb")    # [j, jb, hd]

            def body():
                # ------- phase A DMAs ------------------------------------
                # Every dma_start occupies its issuing sequencer ~0.6-1us in
                # strict FIFO order, and a DMA whose input isn't ready blocks
                # the whole ring behind it.  So: kv-feeding DMAs first on the
                # sync ring (coarse chunks, ~10 issues), wq/wo/x-prefetch
                # next (no input deps), kv_part export last (its sem wait
                # parks the ring until ~25us, when nothing else is pending).
                ctxT_sb = workp.tile([128, DC, M], BF16, name="ctxT_sb",
                                     tag="t12", bufs=3)
                wkp_sb = workp.tile([128, DC, 384], BF16, name="wkp_sb",
                                    tag="t12", bufs=3)
                wvp_sb = workp.tile([128, DC, 384], BF16, name="wvp_sb",
                                    tag="t12", bufs=3)
                # k-feeds first (k chains are the first PE work), then v,
                # then x-prefetch, then wq in COLUMN slices (so qproj ec=0
                # can start before all of wq lands), then wo.  HBM is the
                # scarce resource 0-35us: everything here is ordered by
                # first-use time.
                c03, crest = slice(0, 3), slice(3, 12)
                nc.sync.dma_start(out=wkp_sb[:, c03, :], in_=wkp_r[:, c03, :])
                nc.sync.dma_start(out=ctxT_sb[:, c03, :],
                                  in_=ctxT_r[:, c03, :])
                nc.sync.dma_start(out=wkp_sb[:, crest, :],
                                  in_=wkp_r[:, crest, :])
                nc.sync.dma_start(out=ctxT_sb[:, crest, :],
                                  in_=ctxT_r[:, crest, :])
                nc.sync.dma_start(out=wq_sb[:, :, 0:512],
                                  in_=wq_r[:, :, 0:512])
                nc.sync.dma_start(out=wvp_sb[:, 0:6, :], in_=wvp_r[:, 0:6, :])
                nc.sync.dma_start(out=wvp_sb[:, 6:12, :],
                                  in_=wvp_r[:, 6:12, :])
                xTb_pre = []
                for ib in range(2):
                    xTb = workp.tile([128, DC, 128], BF16, name="xTb",
                                     tag="t3", bufs=6)
                    nc.sync.dma_start(
                        out=xTb[:], in_=xT_r[:, :, ib * 128:(ib + 1) * 128])
                    xTb_pre.append(xTb)
                nc.sync.dma_start(out=wq_sb[:, :, 512:1024],
                                  in_=wq_r[:, :, 512:1024])
                nc.sync.dma_start(out=wq_sb[:, :, 1024:1536],
                                  in_=wq_r[:, :, 1024:1536])

                # ------- kv projection partials (this core's 1/4) ---------
                kT_part = workp.tile([128, 3, M], BF16, name="kT_part",
                                     tag="t3", bufs=6)
                for h in range(3):
                    kps = psp.tile([128, M], F32, name="kps", tag="acc",
                                   bufs=2)
                    for c in range(DC):
                        nc.tensor.matmul(
                            kps[:],
                            lhsT=wkp_sb[:, c, h * 128:(h + 1) * 128],
                            rhs=ctxT_sb[:, c, :],
                            start=(c == 0), stop=(c == DC - 1))
                    nc.scalar.copy(kT_part[:, h, :], kps[:])
                v_part = workp.tile([128, JB, 384], BF16, name="v_part",
                                    tag="t3", bufs=6)
                for jb in range(JB):
                    vps = psp.tile([128, 512], F32, name="vps", tag="acc",
                                   bufs=2)
                    for c in range(DC):
                        nc.tensor.matmul(
                            vps[:, :384],
                            lhsT=ctxT_sb[:, c, jb * 128:(jb + 1) * 128],
                            rhs=wvp_sb[:, c, :],
                            start=(c == 0), stop=(c == DC - 1))
                    nc.scalar.copy(v_part[:, jb, :], vps[:, :384])

                kv_part = dramp.tile([256, 1536], BF16, name="kv_part")
                nc.sync.dma_start(out=kv_part[0:128, :], in_=kT_part[:])
                nc.sync.dma_start(out=kv_part[128:256, :], in_=v_part[:])
                # wo AFTER kv_part: a huge in-flight DMA ahead of kv_part on
                # the shared completion-sem lanes would delay the collective
                nc.sync.dma_start(out=wo_sb[:], in_=wo_r)
                kv_gath = dramp.tile([1024, 1536], BF16, name="kv_gath")
                nc.gpsimd.collective_compute(
                    "AllGather", mybir.AluOpType.bypass,
                    replica_groups=[[0, 1, 2, 3], [4, 5, 6, 7]],
                    ins=[kv_part[:]], outs=[kv_gath[:]])
                # gather-back on the gpsimd (SWDGE) ring: its sem wait on
                # the collective must not block the sync ring, and the next
                # gpsimd work (rowsum adds) isn't needed until attention.
                # Merged into 2 strided DMAs (Q7 desc-gen is ~1us per DMA).
                kT_src = kv_gath[:].rearrange(
                    "(q x p) (a b) -> x p q a b", q=4, x=2, a=3)[0]
                nc.gpsimd.dma_start(
                    out=kT_sb[:].rearrange("p (q a) m -> p q a m", q=4),
                    in_=kT_src)
                v_src = kv_gath[:].rearrange(
                    "(q x p) (a b) -> x p a q b", q=4, x=2, a=4)[1]
                nc.gpsimd.dma_start(
                    out=v_sb[:].rearrange("p a (q b) -> p a q b", q=4),
                    in_=v_src)

                # ------- q pipeline pieces --------------------------------
                def qpipe_proj(ib):
                    """q projection + RMS-norm scale for 128-row block ib.
                    Returns the scaled bf16 q tile (natural layout)."""
                    if ib < len(xTb_pre):
                        xTb = xTb_pre[ib]
                    else:
                        xTb = workp.tile([128, DC, 128], BF16, name="xTb",
                                         tag="t3", bufs=6)
                        nc.sync.dma_start(
                            out=xTb[:],
                            in_=xT_r[:, :, ib * 128:(ib + 1) * 128])
                    qbf = workp.tile([128, H, 128], BF16, name="qbf",
                                     tag="t3", bufs=6)
                    ssq = workp.tile([128, H], F32, name="ssq", tag="tiny",
                                     bufs=12)
                    for ec in range(3):
                        psq = psp.tile([128, 512], F32, name="psq",
                                       tag="proj", bufs=2)
                        for c in range(DC):
                            nc.tensor.matmul(
                                psq[:], lhsT=xTb[:, c, :],
                                rhs=wq_sb[:, c, ec * 512:(ec + 1) * 512],
                                start=(c == 0), stop=(c == DC - 1))
                        scr = workp.tile([128, 512], F32, name="scr",
                                         tag="scr", bufs=2)
                        nc.scalar.copy(
                            qbf[:, ec * 4:(ec + 1) * 4, :],
                            psq[:].rearrange("p (a b) -> p a b", a=4))
                        nc.vector.tensor_mul(
                            scr[:].rearrange("p (a b) -> p a b", a=4),
                            qbf[:, ec * 4:(ec + 1) * 4, :],
                            qbf[:, ec * 4:(ec + 1) * 4, :])
                        nc.vector.tensor_reduce(
                            out=ssq[:, ec * 4:(ec + 1) * 4],
                            in_=scr[:].rearrange("p (a b) -> p a b", a=4),
                            axis=mybir.AxisListType.X, op=ADD)
                    # c = rsqrt(ssq + HD*eps), quake + 2 newton steps (DVE)
                    sse = workp.tile([128, H], F32, name="sse", tag="tiny",
                                     bufs=12)
                    nc.vector.tensor_scalar_add(sse[:], ssq[:], EPSH)
                    yi = workp.tile([128, H], I32, name="yi", tag="tiny",
                                    bufs=12)
                    nc.vector.tensor_scalar(
                        yi[:], sse[:].bitcast(I32), 1, -1,
                        op0=LSR, op1=XOR)
                    nc.vector.tensor_scalar_add(yi[:], yi[:], MAGIC_P1)
                    y = yi[:].bitcast(F32)
                    na = workp.tile([128, H], F32, name="na", tag="tiny",
                                    bufs=12)
                    for _ in range(2):
                        nc.vector.tensor_mul(na[:], sse[:], y)
                        nc.vector.tensor_mul(na[:], na[:], y)
                        nc.vector.tensor_scalar(
                            na[:], na[:], -0.5, 1.5, op0=MULT, op1=ADD)
                        nc.vector.tensor_mul(y, y, na[:])
                    for h in range(H):
                        nc.vector.tensor_scalar_mul(
                            qbf[:, h, :], qbf[:, h, :],
                            yi[:, h:h + 1].bitcast(F32))
                    return qbf

                def qpipe_trans(qbf, bi, qT):
                    """transpose a scaled q tile into qT[:, :, bi-block]."""
                    for t3c in range(3):
                        tps = psp.tile([128, 512], BF16, name="tps",
                                       tag="proj", bufs=2)
                        for cc in range(4):
                            h = t3c * 4 + cc
                            nc.tensor.transpose(
                                tps[:, cc * 128:(cc + 1) * 128],
                                qbf[:, h, :], ident_b)
                        nc.scalar.copy(
                            qT[:, t3c * 4:(t3c + 1) * 4,
                               bi * 128:(bi + 1) * 128],
                            tps[:].rearrange("p (a b) -> p a b", a=4))

                qTs = {}
                oTs = {}
                pend_tr = []                 # (qbf, block)
                proj_left = list(range(NBLK))

                def gi_of(b):
                    return 0 if b < GROUPS[1][0] else (
                        1 if b < GROUPS[2][0] else 2)

                def do_proj():
                    b = proj_left.pop(0)
                    gi = gi_of(b)
                    if b == GROUPS[gi][0]:
                        qTs[gi] = workp.tile([128, H, 512], BF16, name="qT",
                                             tag="t12", bufs=3)
                    pend_tr.append((qpipe_proj(b), b))

                def do_trans():
                    qbf, b = pend_tr.pop(0)
                    gi = gi_of(b)
                    qpipe_trans(qbf, b - GROUPS[gi][0], qTs[gi])

                # ------- prologue: PRO blocks, transposes trail by one ----
                do_proj()
                do_proj()
                for _ in range(PRO - 2):
                    do_trans()
                    do_proj()
                do_trans()
                do_trans()

                # ------- attention, software-pipelined one head deep ------
                def attn_scores(h, gw, qT):
                    """scores + exp for head h; rowsum partials on gpsimd/
                    DVE. Returns (pT, rsum)."""
                    pT = workp.tile([128, JB, 512], BF16, name="pT",
                                    tag="pT", bufs=2)
                    prt = workp.tile([128, 2, 512], BF16, name="prt",
                                     tag="prs", bufs=2)
                    for half in range(2):
                        sT = psp.tile([128, 2, 512], F32, name="sT",
                                      tag="sT", bufs=2)
                        for jj in range(2):
                            jb = half * 2 + jj
                            nc.tensor.matmul(
                                sT[:, jj, :gw],
                                lhsT=kT_sb[:, h, jb * 128:(jb + 1) * 128],
                                rhs=qT[:, h, :gw], start=True, stop=True)
                        nc.scalar.activation(
                            pT[:, half * 2:(half + 1) * 2, :gw],
                            sT[:, :, :gw], EXP)
                        nc.gpsimd.tensor_add(
                            prt[:, half, :gw],
                            pT[:, half * 2, :gw],
                            pT[:, half * 2 + 1, :gw])
                    rsum = workp.tile([128, 512], BF16, name="rsum",
                                      tag="prs2", bufs=2)
                    nc.vector.tensor_add(
                        rsum[:, :gw], prt[:, 0, :gw], prt[:, 1, :gw])
                    return pT, rsum

                def attn_tail(h, gw, pT, rsum, oT):
                    """AV + single rowsum matmul + normalize for head h."""
                    av = psp.tile([128, 512], F32, name="av", tag="acc",
                                  bufs=2)
                    for jb in range(JB):
                        nc.tensor.matmul(
                            av[:, :gw],
                            lhsT=v_sb[:, jb, h * 128:(h + 1) * 128],
                            rhs=pT[:, jb, :gw],
                            start=(jb == 0), stop=(jb == JB - 1))
                    sm = psp.tile([128, 512], F32, name="sm", tag="acc",
                                  bufs=2)
                    nc.tensor.matmul(
                        sm[:, :gw], lhsT=ones_b[:], rhs=rsum[:, :gw],
                        start=True, stop=True)
                    rs = workp.tile([128, 512], F32, name="rs", tag="s2",
                                    bufs=4)
                    nc.vector.reciprocal_approx_fast(rs[:, :gw], sm[:, :gw])
                    nc.vector.tensor_mul(
                        oT[:, h, :gw], av[:, :gw], rs[:, :gw])

                def outproj_chunk(ib, bi, ec, oT):
                    sl = slice(ec * 512, (ec + 1) * 512)
                    po = psp.tile([128, 512], F32, name="po", tag="proj",
                                  bufs=2)
                    for hh in range(H):
                        nc.tensor.matmul(
                            po[:],
                            lhsT=oT[:, hh, bi * 128:(bi + 1) * 128],
                            rhs=wo_sb[:, hh, sl],
                            start=(hh == 0), stop=(hh == H - 1))
                    och = workp.tile([128, 512], F32, name="och", tag="s2",
                                     bufs=4)
                    nc.vector.tensor_copy(och[:], po[:])
                    nc.sync.dma_start(
                        out=out_d[ib * 128:(ib + 1) * 128, sl], in_=och[:])

                pend = None
                for gi, (g0b, gn) in enumerate(GROUPS):
                    gw = gn * 128
                    oTs[gi] = workp.tile([128, H, 512], BF16, name="oT",
                                         tag="oT", bufs=2)
                    op_left = []
                    if gi > 0:
                        pg0, pgn = GROUPS[gi - 1]
                        op_left = [(pg0 + bi, bi, ec)
                                   for bi in range(pgn) for ec in range(3)]
                    for h in range(H):
                        if h == 0 and pend is not None:
                            # finish prev group before its outproj fillers
                            attn_tail(*pend)
                            pend = None
                        cur = attn_scores(h, gw, qTs[gi])
                        # PE filler between scores(h) and tail(h-1): covers
                        # the exp+gpsimd latency of head h
                        if op_left:
                            ib, bi, ec = op_left.pop(0)
                            outproj_chunk(ib, bi, ec, oTs[gi - 1])
                        elif gi == 0:
                            if h % 2 == 1 and proj_left:
                                do_proj()
                            elif pend_tr:
                                do_trans()
                        if pend is not None:
                            attn_tail(*pend)
                        pend = (h, gw, cur[0], cur[1], oTs[gi])
                    while op_left:
                        ib, bi, ec = op_left.pop(0)
                        outproj_chunk(ib, bi, ec, oTs[gi - 1])
                    while pend_tr:
                        do_trans()
                attn_tail(*pend)
                # epilogue: outproj of last group
                lg0, lgn = GROUPS[-1]
                for bi in range(lgn):
                    for ec in range(3):
                        outproj_chunk(lg0 + bi, bi, ec, oTs[len(GROUPS) - 1])

            if reps == 1:
                body()
            else:
                with tc.For_i(0, reps, 1):
                    body()
    nc.finalize()
    return nc


def kernel(x, context, wq, bq, wkv, bkv, wo, bo, q_norm_scale):
    x = np.asarray(x, dtype=np.float32)
    context = np.asarray(context, dtype=np.float32)
    bf = ml_dtypes.bfloat16

    if "nc" not in _cache:
        _cache["nc"] = _build()
    nc = _cache["nc"]

    scale_t = np.tile(np.asarray(q_norm_scale, np.float32), H)      # [D]
    wkv_f = np.asarray(wkv, np.float32)
    wk_b = (wkv_f[:, :D] * scale_t[None, :]).astype(bf)
    wv_b = np.ascontiguousarray(wkv_f[:, D:]).astype(bf)
    wq_b = np.asarray(wq, np.float32).astype(bf)
    wo_b = np.asarray(wo, np.float32).astype(bf)

    xp = np.zeros((B, CPB * RPC, D), np.float32)
    xp[:, :N] = x
    ctxT_b = [np.ascontiguousarray(context[b].T).astype(bf) for b in range(B)]

    in_maps = []
    for core in range(NCORES):
        b, q = divmod(core, CPB)
        xT = np.ascontiguousarray(xp[b, q * RPC:(q + 1) * RPC].T).astype(bf)
        in_maps.append({
            "xT": xT,
            "ctxT": ctxT_b[b],
            "wq": wq_b,
            "wkp": np.ascontiguousarray(wk_b[:, q * 384:(q + 1) * 384]),
            "wvp": np.ascontiguousarray(wv_b[:, q * 384:(q + 1) * 384]),
            "wo": wo_b,
        })

    res = bass_utils.run_bass_kernel_spmd(
        nc, in_maps, core_ids=list(range(NCORES)), trace=TRACE)
    _cache["last_results"] = res

    out = np.empty((B, N, D), np.float32)
    for b in range(B):
        cat = np.concatenate(
            [res.results[b * CPB + q]["out"] for q in range(CPB)], axis=0)
        out[b] = cat[:N]
    return out


# revision 19
# speedup vs baseline: 1.0323x; 1.0323x over previous
"""Cross-attention (q-norm variant) Trainium2 Bass kernel, v4.

Sharding: batch (2) x row-quarters (4) -> 8 cores, data-parallel over the
query sequence. Each core handles 1408 query rows (5376 padded to 5632 per
batch) of ONE batch, with that batch's context replicated. kv projection is
computed 1/4-per-core within each batch group and all-gathered.

v4 changes over v2 (all scheduling / engine-balance; math identical):
  - DMA issue is sequencer-FIFO-ordered with ~0.6-1us issue cost and
    head-of-line sem blocking, so DMAs are laid out per-ring by priority:
    sync ring gets the kv-feeding loads (coarse chunks), then wq/wo/x
    prefetches, then kv_part export (whose sem wait parks the ring at a
    point where nothing else is pending); the collective gather-back DMAs
    live on the gpsimd SWDGE ring whose next work (rowsum adds) isn't
    needed until attention anyway.  The collective now dispatches at
    ~27us instead of ~64us.
  - softmax row sums: the 4 ones-matmuls per head are replaced by
    gpsimd/DVE partial sums over the 4 j-blocks (pT0+pT1, pT2+pT3 on
    gpsimd, final add on DVE) + ONE ones-matmul -> 3/4 of that PE time
    moves to otherwise-idle engines.
  - attention is software-pipelined one head deep: head h's scores+exp are
    emitted before head h-1's AV/rowsum/normalize, so the exp latency is
    covered by real PE work instead of stalling.
  - AV matmuls run before the rowsum matmul (rowsum partials are still in
    flight on gpsimd while AV streams).
  - prologue projects PRO=7 q row-blocks before attention group 0 so the
    PE stays busy through the kv all-gather (~35us payload transfer).
  - outproj PSUM->SBUF copies moved from ACT to DVE (ACT keeps only exp
    and q-proj copies).

Host-side prep (numpy): cast weights to bf16, fold q_norm_scale into the
k-half of wkv, transpose x and context. Biases are structurally zero in
this problem and are dropped.
"""

import sys
import numpy as np

for _p in ("/opt/trn_rl_repo",):
    if _p not in sys.path:
        sys.path.insert(0, _p)

import ml_dtypes
import concourse.bass as bass
import concourse.tile as tile
from concourse import bacc, mybir
from concourse import bass_utils
from concourse.masks import make_identity

F32 = mybir.dt.float32
BF16 = mybir.dt.bfloat16
I32 = mybir.dt.int32
EXP = mybir.ActivationFunctionType.Exp
MULT = mybir.AluOpType.mult
ADD = mybir.AluOpType.add
LSR = mybir.AluOpType.logical_shift_right
XOR = mybir.AluOpType.bitwise_xor

B, N, D, M, H, HD = 2, 5376, 1536, 512, 12, 128
EPS = 1e-6
EPSH = float(HD * EPS)
NCORES = 8
CPB = 4            # cores per batch
RPC = 1408         # padded rows per core  (4*1408 = 5632 >= 5376)
NBLK = RPC // 128  # 11
DC = D // 128      # 12 contraction chunks
JB = M // 128      # 4 context row blocks
GROUPS = [(0, 4), (4, 4), (8, 3)]   # (start block, #blocks)
PRO = 11           # q row-blocks projected before attention starts
                   # (all of them: covers the collective's peer-skew tail)
MAGIC_P1 = 0x5F3759E0  # quake rsqrt magic + 1

TRACE = False

_cache = {}


def _build(reps=1):
    nc = bacc.Bacc(
        "TRN2", target_bir_lowering=False, debug=False, num_devices=NCORES
    )
    xT_d = nc.dram_tensor("xT", [D, RPC], BF16, kind="ExternalInput").ap()
    ctxT_d = nc.dram_tensor("ctxT", [D, M], BF16, kind="ExternalInput").ap()
    wq_d = nc.dram_tensor("wq", [D, D], BF16, kind="ExternalInput").ap()
    # per-core slices: 3 k-heads / 384 v-cols per core; kv is all-gathered
    wkp_d = nc.dram_tensor("wkp", [D, 384], BF16, kind="ExternalInput").ap()
    wvp_d = nc.dram_tensor("wvp", [D, 384], BF16, kind="ExternalInput").ap()
    wo_d = nc.dram_tensor("wo", [D, D], BF16, kind="ExternalInput").ap()
    out_d = nc.dram_tensor("out", [RPC, D], F32, kind="ExternalOutput").ap()

    xT_r = xT_d.rearrange("(c p) n -> p c n", p=128)      # [128, DC, RPC]
    ctxT_r = ctxT_d.rearrange("(c p) m -> p c m", p=128)  # [128, DC, M]
    wq_r = wq_d.rearrange("(c p) n -> p c n", p=128)
    wkp_r = wkp_d.rearrange("(c p) n -> p c n", p=128)
    wvp_r = wvp_d.rearrange("(c p) n -> p c n", p=128)
    wo_r = wo_d.rearrange("(c p) n -> p c n", p=128)

    with tile.TileContext(nc) as tc:
        with (
            tc.tile_pool(name="const", bufs=1) as constp,
            tc.tile_pool(name="wts", bufs=1) as wtp,
            tc.tile_pool(name="work", bufs=2) as workp,
            tc.tile_pool(name="dram", bufs=1, space="DRAM") as dramp,
            tc.tile_pool(name="ps", bufs=2, space="PSUM") as psp,
        ):
            ident_b = constp.tile([128, 128], BF16, name="ident_b")
            make_identity(nc, ident_b)
            ones_b = constp.tile([128, 128], BF16, name="ones_b")
            nc.vector.memset(ones_b[:], 1.0)

            wq_sb = wtp.tile([128, DC, D], BF16, name="wq_sb")
            wo_sb = wtp.tile([128, DC, D], BF16, name="wo_sb")
            kT_sb = wtp.tile([128, H, M], BF16, name="kT_sb")   # [d, h, j]
            v_sb = wtp.tile([128, JB, D], BF16, name="v_sb")    # [j, jb, hd]

            def body():
                # ------- phase A DMAs ------------------------------------
                # Every dma_start occupies its issuing sequencer ~0.6-1us in
                # strict FIFO order, and a DMA whose input isn't ready blocks
                # the whole ring behind it.  So: kv-feeding DMAs first on the
                # sync ring (coarse chunks, ~10 issues), wq/wo/x-prefetch
                # next (no input deps), kv_part export last (its sem wait
                # parks the ring until ~25us, when nothing else is pending).
                ctxT_sb = workp.tile([128, DC, M], BF16, name="ctxT_sb",
                                     tag="t12", bufs=3)
                wkp_sb = workp.tile([128, DC, 384], BF16, name="wkp_sb",
                                    tag="t12", bufs=3)
                wvp_sb = workp.tile([128, DC, 384], BF16, name="wvp_sb",
                                    tag="t12", bufs=3)
                # k-feeds first (k chains are the first PE work), then v,
                # then x-prefetch, then wq in COLUMN slices (so qproj ec=0
                # can start before all of wq lands), then wo.  HBM is the
                # scarce resource 0-35us: everything here is ordered by
                # first-use time.
                for i in range(4):
                    c3 = slice(3 * i, 3 * i + 3)
                    nc.sync.dma_start(out=wkp_sb[:, c3, :],
                                      in_=wkp_r[:, c3, :])
                    nc.sync.dma_start(out=ctxT_sb[:, c3, :],
                                      in_=ctxT_r[:, c3, :])
                nc.sync.dma_start(out=wq_sb[:, :, 0:512],
                                  in_=wq_r[:, :, 0:512])
                nc.sync.dma_start(out=wvp_sb[:, 0:6, :], in_=wvp_r[:, 0:6, :])
                nc.sync.dma_start(out=wvp_sb[:, 6:12, :],
                                  in_=wvp_r[:, 6:12, :])
                xTb_pre = []
                for ib in range(2):
                    xTb = workp.tile([128, DC, 128], BF16, name="xTb",
                                     tag="t3", bufs=6)
                    nc.sync.dma_start(
                        out=xTb[:], in_=xT_r[:, :, ib * 128:(ib + 1) * 128])
                    xTb_pre.append(xTb)
                nc.sync.dma_start(out=wq_sb[:, :, 512:1024],
                                  in_=wq_r[:, :, 512:1024])
                nc.sync.dma_start(out=wq_sb[:, :, 1024:1536],
                                  in_=wq_r[:, :, 1024:1536])

                # ------- kv projection partials (this core's 1/4) ---------
                kT_part = workp.tile([128, 3, M], BF16, name="kT_part",
                                     tag="t3", bufs=6)
                for h in range(3):
                    kps = psp.tile([128, M], F32, name="kps", tag="acc",
                                   bufs=2)
                    for c in range(DC):
                        nc.tensor.matmul(
                            kps[:],
                            lhsT=wkp_sb[:, c, h * 128:(h + 1) * 128],
                            rhs=ctxT_sb[:, c, :],
                            start=(c == 0), stop=(c == DC - 1))
                    nc.scalar.copy(kT_part[:, h, :], kps[:])
                v_part = workp.tile([128, JB, 384], BF16, name="v_part",
                                    tag="t3", bufs=6)
                for jb in range(JB):
                    vps = psp.tile([128, 512], F32, name="vps", tag="acc",
                                   bufs=2)
                    for c in range(DC):
                        nc.tensor.matmul(
                            vps[:, :384],
                            lhsT=ctxT_sb[:, c, jb * 128:(jb + 1) * 128],
                            rhs=wvp_sb[:, c, :],
                            start=(c == 0), stop=(c == DC - 1))
                    nc.scalar.copy(v_part[:, jb, :], vps[:, :384])

                kv_part = dramp.tile([256, 1536], BF16, name="kv_part")
                nc.sync.dma_start(out=kv_part[0:128, :], in_=kT_part[:])
                nc.sync.dma_start(out=kv_part[128:256, :], in_=v_part[:])
                # wo AFTER kv_part: a huge in-flight DMA ahead of kv_part on
                # the shared completion-sem lanes would delay the collective
                nc.sync.dma_start(out=wo_sb[:], in_=wo_r)
                kv_gath = dramp.tile([1024, 1536], BF16, name="kv_gath")
                nc.gpsimd.collective_compute(
                    "AllGather", mybir.AluOpType.bypass,
                    replica_groups=[[0, 1, 2, 3], [4, 5, 6, 7]],
                    ins=[kv_part[:]], outs=[kv_gath[:]])
                # gather-back on the gpsimd (SWDGE) ring: its sem wait on
                # the collective must not block the sync ring, and the next
                # gpsimd work (rowsum adds) isn't needed until attention.
                # Merged into 2 strided DMAs (Q7 desc-gen is ~1us per DMA).
                kT_src = kv_gath[:].rearrange(
                    "(q x p) (a b) -> x p q a b", q=4, x=2, a=3)[0]
                nc.gpsimd.dma_start(
                    out=kT_sb[:].rearrange("p (q a) m -> p q a m", q=4),
                    in_=kT_src)
                v_src = kv_gath[:].rearrange(
                    "(q x p) (a b) -> x p a q b", q=4, x=2, a=4)[1]
                nc.gpsimd.dma_start(
                    out=v_sb[:].rearrange("p a (q b) -> p a q b", q=4),
                    in_=v_src)

                # ------- q pipeline pieces --------------------------------
                def qpipe_proj(ib):
                    """q projection + RMS-norm scale for 128-row block ib.
                    Returns the scaled bf16 q tile (natural layout)."""
                    if ib < len(xTb_pre):
                        xTb = xTb_pre[ib]
                    else:
                        xTb = workp.tile([128, DC, 128], BF16, name="xTb",
                                         tag="t3", bufs=6)
                        nc.sync.dma_start(
                            out=xTb[:],
                            in_=xT_r[:, :, ib * 128:(ib + 1) * 128])
                    qbf = workp.tile([128, H, 128], BF16, name="qbf",
                                     tag="t3", bufs=6)
                    ssq = workp.tile([128, H], F32, name="ssq", tag="tiny",
                                     bufs=12)
                    for ec in range(3):
                        psq = psp.tile([128, 512], F32, name="psq",
                                       tag="proj", bufs=2)
                        for c in range(DC):
                            nc.tensor.matmul(
                                psq[:], lhsT=xTb[:, c, :],
                                rhs=wq_sb[:, c, ec * 512:(ec + 1) * 512],
                                start=(c == 0), stop=(c == DC - 1))
                        scr = workp.tile([128, 512], F32, name="scr",
                                         tag="scr", bufs=2)
                        nc.scalar.copy(
                            qbf[:, ec * 4:(ec + 1) * 4, :],
                            psq[:].rearrange("p (a b) -> p a b", a=4))
                        nc.vector.tensor_mul(
                            scr[:].rearrange("p (a b) -> p a b", a=4),
                            qbf[:, ec * 4:(ec + 1) * 4, :],
                            qbf[:, ec * 4:(ec + 1) * 4, :])
                        nc.vector.tensor_reduce(
                            out=ssq[:, ec * 4:(ec + 1) * 4],
                            in_=scr[:].rearrange("p (a b) -> p a b", a=4),
                            axis=mybir.AxisListType.X, op=ADD)
                    # c = rsqrt(ssq + HD*eps), quake + 2 newton steps (DVE)
                    sse = workp.tile([128, H], F32, name="sse", tag="tiny",
                                     bufs=12)
                    nc.vector.tensor_scalar_add(sse[:], ssq[:], EPSH)
                    yi = workp.tile([128, H], I32, name="yi", tag="tiny",
                                    bufs=12)
                    nc.vector.tensor_scalar(
                        yi[:], sse[:].bitcast(I32), 1, -1,
                        op0=LSR, op1=XOR)
                    nc.vector.tensor_scalar_add(yi[:], yi[:], MAGIC_P1)
                    y = yi[:].bitcast(F32)
                    na = workp.tile([128, H], F32, name="na", tag="tiny",
                                    bufs=12)
                    for _ in range(2):
                        nc.vector.tensor_mul(na[:], sse[:], y)
                        nc.vector.tensor_mul(na[:], na[:], y)
                        nc.vector.tensor_scalar(
                            na[:], na[:], -0.5, 1.5, op0=MULT, op1=ADD)
                        nc.vector.tensor_mul(y, y, na[:])
                    for h in range(H):
                        nc.vector.tensor_scalar_mul(
                            qbf[:, h, :], qbf[:, h, :],
                            yi[:, h:h + 1].bitcast(F32))
                    return qbf

                def qpipe_trans(qbf, bi, qT):
                    """transpose a scaled q tile into qT[:, :, bi-block]."""
                    for t3c in range(3):
                        tps = psp.tile([128, 512], BF16, name="tps",
                                       tag="proj", bufs=2)
                        for cc in range(4):
                            h = t3c * 4 + cc
                            nc.tensor.transpose(
                                tps[:, cc * 128:(cc + 1) * 128],
                                qbf[:, h, :], ident_b)
                        nc.scalar.copy(
                            qT[:, t3c * 4:(t3c + 1) * 4,
                               bi * 128:(bi + 1) * 128],
                            tps[:].rearrange("p (a b) -> p a b", a=4))

                qTs = {}
                oTs = {}
                pend_tr = []                 # (qbf, block)
                proj_left = list(range(NBLK))

                def gi_of(b):
                    return 0 if b < GROUPS[1][0] else (
                        1 if b < GROUPS[2][0] else 2)

                def do_proj():
                    b = proj_left.pop(0)
                    gi = gi_of(b)
                    if b == GROUPS[gi][0]:
                        qTs[gi] = workp.tile([128, H, 512], BF16, name="qT",
                                             tag="t12", bufs=3)
                    pend_tr.append((qpipe_proj(b), b))

                def do_trans():
                    qbf, b = pend_tr.pop(0)
                    gi = gi_of(b)
                    qpipe_trans(qbf, b - GROUPS[gi][0], qTs[gi])

                # ------- prologue: PRO blocks, transposes trail by one ----
                do_proj()
                do_proj()
                for _ in range(PRO - 2):
                    do_trans()
                    do_proj()
                do_trans()
                do_trans()

                # ------- attention, software-pipelined one head deep ------
                def attn_scores(h, gw, qT):
                    """scores + exp for head h; rowsum partials on gpsimd/
                    DVE. Returns (pT, rsum)."""
                    pT = workp.tile([128, JB, 512], BF16, name="pT",
                                    tag="pT", bufs=2)
                    prt = workp.tile([128, 2, 512], BF16, name="prt",
                                     tag="prs", bufs=2)
                    for half in range(2):
                        sT = psp.tile([128, 2, 512], F32, name="sT",
                                      tag="sT", bufs=2)
                        for jj in range(2):
                            jb = half * 2 + jj
                            nc.tensor.matmul(
                                sT[:, jj, :gw],
                                lhsT=kT_sb[:, h, jb * 128:(jb + 1) * 128],
                                rhs=qT[:, h, :gw], start=True, stop=True)
                        nc.scalar.activation(
                            pT[:, half * 2:(half + 1) * 2, :gw],
                            sT[:, :, :gw], EXP)
                        nc.vector.tensor_add(
                            prt[:, half, :gw],
                            pT[:, half * 2, :gw],
                            pT[:, half * 2 + 1, :gw])
                    rsum = workp.tile([128, 512], BF16, name="rsum",
                                      tag="prs2", bufs=2)
                    nc.vector.tensor_add(
                        rsum[:, :gw], prt[:, 0, :gw], prt[:, 1, :gw])
                    return pT, rsum

                def attn_tail(h, gw, pT, rsum, oT):
                    """AV + single rowsum matmul + normalize for head h."""
                    av = psp.tile([128, 512], F32, name="av", tag="acc",
                                  bufs=2)
                    for jb in range(JB):
                        nc.tensor.matmul(
                            av[:, :gw],
                            lhsT=v_sb[:, jb, h * 128:(h + 1) * 128],
                            rhs=pT[:, jb, :gw],
                            start=(jb == 0), stop=(jb == JB - 1))
                    sm = psp.tile([128, 512], F32, name="sm", tag="acc",
                                  bufs=2)
                    nc.tensor.matmul(
                        sm[:, :gw], lhsT=ones_b[:], rhs=rsum[:, :gw],
                        start=True, stop=True)
                    rs = workp.tile([128, 512], F32, name="rs", tag="s2",
                                    bufs=4)
                    nc.vector.reciprocal_approx_fast(rs[:, :gw], sm[:, :gw])
                    nc.vector.tensor_mul(
                        oT[:, h, :gw], av[:, :gw], rs[:, :gw])

                def outproj_chunk(ib, bi, ec, oT):
                    sl = slice(ec * 512, (ec + 1) * 512)
                    po = psp.tile([128, 512], F32, name="po", tag="proj",
                                  bufs=2)
                    for hh in range(H):
                        nc.tensor.matmul(
                            po[:],
                            lhsT=oT[:, hh, bi * 128:(bi + 1) * 128],
                            rhs=wo_sb[:, hh, sl],
                            start=(hh == 0), stop=(hh == H - 1))
                    och = workp.tile([128, 512], F32, name="och", tag="s2",
                                     bufs=4)
                    nc.vector.tensor_copy(och[:], po[:])
                    nc.sync.dma_start(
                        out=out_d[ib * 128:(ib + 1) * 128, sl], in_=och[:])

                pend = None
                for gi, (g0b, gn) in enumerate(GROUPS):
                    gw = gn * 128
                    oTs[gi] = workp.tile([128, H, 512], BF16, name="oT",
                                         tag="oT", bufs=2)
                    op_left = []
                    if gi > 0:
                        pg0, pgn = GROUPS[gi - 1]
                        op_left = [(pg0 + bi, bi, ec)
                                   for bi in range(pgn) for ec in range(3)]
                    for h in range(H):
                        if h == 0 and pend is not None:
                            # finish prev group before its outproj fillers
                            attn_tail(*pend)
                            pend = None
                        cur = attn_scores(h, gw, qTs[gi])
                        # PE filler between scores(h) and tail(h-1): covers
                        # the exp+rowsum latency of head h.  Skipped at h==0
                        # so the first chunk never waits on the just-flushed
                        # last head of the previous group's oT.
                        if op_left and h > 0:
                            ib, bi, ec = op_left.pop(0)
                            outproj_chunk(ib, bi, ec, oTs[gi - 1])
                        elif gi == 0:
                            if h % 2 == 1 and proj_left:
                                do_proj()
                            elif pend_tr:
                                do_trans()
                        if pend is not None:
                            attn_tail(*pend)
                        pend = (h, gw, cur[0], cur[1], oTs[gi])
                    while op_left:
                        ib, bi, ec = op_left.pop(0)
                        outproj_chunk(ib, bi, ec, oTs[gi - 1])
                    while pend_tr:
                        do_trans()
                attn_tail(*pend)
                # epilogue: outproj of last group
                lg0, lgn = GROUPS[-1]
                for bi in range(lgn):
                    for ec in range(3):
                        outproj_chunk(lg0 + bi, bi, ec, oTs[len(GROUPS) - 1])

            if reps == 1:
                body()
            else:
                with tc.For_i(0, reps, 1):
                    body()
    nc.finalize()
    return nc


def kernel(x, context, wq, bq, wkv, bkv, wo, bo, q_norm_scale):
    x = np.asarray(x, dtype=np.float32)
    context = np.asarray(context, dtype=np.float32)
    bf = ml_dtypes.bfloat16

    if "nc" not in _cache:
        _cache["nc"] = _build()
    nc = _cache["nc"]

    scale_t = np.tile(np.asarray(q_norm_scale, np.float32), H)      # [D]
    wkv_f = np.asarray(wkv, np.float32)
    wk_b = (wkv_f[:, :D] * scale_t[None, :]).astype(bf)
    wv_b = np.ascontiguousarray(wkv_f[:, D:]).astype(bf)
    wq_b = np.asarray(wq, np.float32).astype(bf)
    wo_b = np.asarray(wo, np.float32).astype(bf)

    xp = np.zeros((B, CPB * RPC, D), np.float32)
    xp[:, :N] = x
    ctxT_b = [np.ascontiguousarray(context[b].T).astype(bf) for b in range(B)]

    in_maps = []
    for core in range(NCORES):
        b, q = divmod(core, CPB)
        xT = np.ascontiguousarray(xp[b, q * RPC:(q + 1) * RPC].T).astype(bf)
        in_maps.append({
            "xT": xT,
            "ctxT": ctxT_b[b],
            "wq": wq_b,
            "wkp": np.ascontiguousarray(wk_b[:, q * 384:(q + 1) * 384]),
            "wvp": np.ascontiguousarray(wv_b[:, q * 384:(q + 1) * 384]),
            "wo": wo_b,
        })

    res = bass_utils.run_bass_kernel_spmd(
        nc, in_maps, core_ids=list(range(NCORES)), trace=TRACE)
    _cache["last_results"] = res

    out = np.empty((B, N, D), np.float32)
    for b in range(B):
        cat = np.concatenate(
            [res.results[b * CPB + q]["out"] for q in range(CPB)], axis=0)
        out[b] = cat[:N]
    return out


# revision 21
# speedup vs baseline: 1.1376x; 1.1020x over previous
"""Cross-attention (q-norm variant) Trainium2 Bass kernel, v4.

Sharding: batch (2) x row-quarters (4) -> 8 cores, data-parallel over the
query sequence. Each core handles 1408 query rows (5376 padded to 5632 per
batch) of ONE batch, with that batch's context replicated. kv projection is
computed 1/4-per-core within each batch group and all-gathered.

v4 changes over v2 (all scheduling / engine-balance; math identical):
  - DMA issue is sequencer-FIFO-ordered with ~0.6-1us issue cost and
    head-of-line sem blocking, so DMAs are laid out per-ring by priority:
    sync ring gets the kv-feeding loads (coarse chunks), then wq/wo/x
    prefetches, then kv_part export (whose sem wait parks the ring at a
    point where nothing else is pending); the collective gather-back DMAs
    live on the gpsimd SWDGE ring whose next work (rowsum adds) isn't
    needed until attention anyway.  The collective now dispatches at
    ~27us instead of ~64us.
  - softmax row sums: the 4 ones-matmuls per head are replaced by
    gpsimd/DVE partial sums over the 4 j-blocks (pT0+pT1, pT2+pT3 on
    gpsimd, final add on DVE) + ONE ones-matmul -> 3/4 of that PE time
    moves to otherwise-idle engines.
  - attention is software-pipelined one head deep: head h's scores+exp are
    emitted before head h-1's AV/rowsum/normalize, so the exp latency is
    covered by real PE work instead of stalling.
  - AV matmuls run before the rowsum matmul (rowsum partials are still in
    flight on gpsimd while AV streams).
  - prologue projects PRO=7 q row-blocks before attention group 0 so the
    PE stays busy through the kv all-gather (~35us payload transfer).
  - outproj PSUM->SBUF copies moved from ACT to DVE (ACT keeps only exp
    and q-proj copies).

Host-side prep (numpy): cast weights to bf16, fold q_norm_scale into the
k-half of wkv, transpose x and context. Biases are structurally zero in
this problem and are dropped.
"""

import sys
import numpy as np

for _p in ("/opt/trn_rl_repo",):
    if _p not in sys.path:
        sys.path.insert(0, _p)

import ml_dtypes
import concourse.bass as bass
import concourse.tile as tile
from concourse import bacc, mybir
from concourse import bass_utils
from concourse.masks import make_identity

F32 = mybir.dt.float32
BF16 = mybir.dt.bfloat16
I32 = mybir.dt.int32
EXP = mybir.ActivationFunctionType.Exp
MULT = mybir.AluOpType.mult
ADD = mybir.AluOpType.add
LSR = mybir.AluOpType.logical_shift_right
XOR = mybir.AluOpType.bitwise_xor

B, N, D, M, H, HD = 2, 5376, 1536, 512, 12, 128
EPS = 1e-6
EPSH = float(HD * EPS)
NCORES = 8
CPB = 4            # cores per batch
RPC = 1408         # padded rows per core  (4*1408 = 5632 >= 5376)
NBLK = RPC // 128  # 11
DC = D // 128      # 12 contraction chunks
JB = M // 128      # 4 context row blocks
GROUPS = [(0, 4), (4, 4), (8, 3)]   # (start block, #blocks)
PRO = 11           # q row-blocks projected before attention starts
                   # (all of them: covers the collective's peer-skew tail)
MAGIC_P1 = 0x5F3759E0  # quake rsqrt magic + 1

TRACE = False

_cache = {}


def _build(reps=1):
    nc = bacc.Bacc(
        "TRN2", target_bir_lowering=False, debug=False, num_devices=NCORES
    )
    xT_d = nc.dram_tensor("xT", [D, RPC], BF16, kind="ExternalInput").ap()
    ctxT_d = nc.dram_tensor("ctxT", [D, M], BF16, kind="ExternalInput").ap()
    wq_d = nc.dram_tensor("wq", [D, D], BF16, kind="ExternalInput").ap()
    # per-core slices: 3 k-heads / 384 v-cols per core; kv is all-gathered
    wkp_d = nc.dram_tensor("wkp", [D, 384], BF16, kind="ExternalInput").ap()
    wvp_d = nc.dram_tensor("wvp", [D, 384], BF16, kind="ExternalInput").ap()
    wo_d = nc.dram_tensor("wo", [D, D], BF16, kind="ExternalInput").ap()
    out_d = nc.dram_tensor("out", [RPC, D], F32, kind="ExternalOutput").ap()

    xT_r = xT_d.rearrange("(c p) n -> p c n", p=128)      # [128, DC, RPC]
    ctxT_r = ctxT_d.rearrange("(c p) m -> p c m", p=128)  # [128, DC, M]
    wq_r = wq_d.rearrange("(c p) n -> p c n", p=128)
    wkp_r = wkp_d.rearrange("(c p) n -> p c n", p=128)
    wvp_r = wvp_d.rearrange("(c p) n -> p c n", p=128)
    wo_r = wo_d.rearrange("(c p) n -> p c n", p=128)

    with tile.TileContext(nc) as tc:
        with (
            tc.tile_pool(name="const", bufs=1) as constp,
            tc.tile_pool(name="wts", bufs=1) as wtp,
            tc.tile_pool(name="work", bufs=2) as workp,
            tc.tile_pool(name="dram", bufs=1, space="DRAM") as dramp,
            tc.tile_pool(name="ps", bufs=2, space="PSUM") as psp,
        ):
            ident_b = constp.tile([128, 128], BF16, name="ident_b")
            make_identity(nc, ident_b)
            ones_b = constp.tile([128, 128], BF16, name="ones_b")
            nc.vector.memset(ones_b[:], 1.0)

            wq_sb = wtp.tile([128, DC, D], BF16, name="wq_sb")
            wo_sb = wtp.tile([128, DC, D], BF16, name="wo_sb")
            kT_sb = wtp.tile([128, H, M], BF16, name="kT_sb")   # [d, h, j]
            v_sb = wtp.tile([128, JB, D], BF16, name="v_sb")    # [j, jb, hd]

            def body():
                # ------- phase A DMAs ------------------------------------
                # Every dma_start occupies its issuing sequencer ~0.6-1us in
                # strict FIFO order, and a DMA whose input isn't ready blocks
                # the whole ring behind it.  So: kv-feeding DMAs first on the
                # sync ring (coarse chunks, ~10 issues), wq/wo/x-prefetch
                # next (no input deps), kv_part export last (its sem wait
                # parks the ring until ~25us, when nothing else is pending).
                ctxT_sb = workp.tile([128, DC, M], BF16, name="ctxT_sb",
                                     tag="t12", bufs=3)
                wkp_sb = workp.tile([128, DC, 384], BF16, name="wkp_sb",
                                    tag="t12", bufs=3)
                wvp_sb = workp.tile([128, DC, 384], BF16, name="wvp_sb",
                                    tag="t12", bufs=3)
                # k-feeds first (k chains are the first PE work), then v,
                # then x-prefetch, then wq in COLUMN slices (so qproj ec=0
                # can start before all of wq lands), then wo.  HBM is the
                # scarce resource 0-35us: everything here is ordered by
                # first-use time.
                for i in range(4):
                    c3 = slice(3 * i, 3 * i + 3)
                    nc.sync.dma_start(out=wkp_sb[:, c3, :],
                                      in_=wkp_r[:, c3, :])
                    nc.sync.dma_start(out=ctxT_sb[:, c3, :],
                                      in_=ctxT_r[:, c3, :])
                nc.sync.dma_start(out=wq_sb[:, :, 0:512],
                                  in_=wq_r[:, :, 0:512])
                nc.sync.dma_start(out=wvp_sb[:, 0:6, :], in_=wvp_r[:, 0:6, :])
                nc.sync.dma_start(out=wvp_sb[:, 6:12, :],
                                  in_=wvp_r[:, 6:12, :])
                xTb_pre = []
                for ib in range(2):
                    xTb = workp.tile([128, DC, 128], BF16, name="xTb",
                                     tag="t3", bufs=8)
                    nc.sync.dma_start(
                        out=xTb[:], in_=xT_r[:, :, ib * 128:(ib + 1) * 128])
                    xTb_pre.append(xTb)
                nc.sync.dma_start(out=wq_sb[:, :, 512:1024],
                                  in_=wq_r[:, :, 512:1024])
                nc.sync.dma_start(out=wq_sb[:, :, 1024:1536],
                                  in_=wq_r[:, :, 1024:1536])

                # ------- kv projection partials (this core's 1/4) ---------
                kT_part = workp.tile([128, 3, M], BF16, name="kT_part",
                                     tag="t3", bufs=8)
                for h in range(3):
                    kps = psp.tile([128, M], F32, name="kps", tag="acc",
                                   bufs=2)
                    for c in range(DC):
                        nc.tensor.matmul(
                            kps[:],
                            lhsT=wkp_sb[:, c, h * 128:(h + 1) * 128],
                            rhs=ctxT_sb[:, c, :],
                            start=(c == 0), stop=(c == DC - 1))
                    nc.scalar.copy(kT_part[:, h, :], kps[:])
                v_part = workp.tile([128, JB, 384], BF16, name="v_part",
                                    tag="t3", bufs=8)
                for jb in range(JB):
                    vps = psp.tile([128, 512], F32, name="vps", tag="acc",
                                   bufs=2)
                    for c in range(DC):
                        nc.tensor.matmul(
                            vps[:, :384],
                            lhsT=ctxT_sb[:, c, jb * 128:(jb + 1) * 128],
                            rhs=wvp_sb[:, c, :],
                            start=(c == 0), stop=(c == DC - 1))
                    nc.scalar.copy(v_part[:, jb, :], vps[:, :384])

                kv_part = dramp.tile([256, 1536], BF16, name="kv_part")
                nc.sync.dma_start(out=kv_part[0:128, :], in_=kT_part[:])
                nc.sync.dma_start(out=kv_part[128:256, :], in_=v_part[:])
                # wo AFTER kv_part: a huge in-flight DMA ahead of kv_part on
                # the shared completion-sem lanes would delay the collective
                nc.sync.dma_start(out=wo_sb[:], in_=wo_r)
                kv_gath = dramp.tile([1024, 1536], BF16, name="kv_gath")
                nc.gpsimd.collective_compute(
                    "AllGather", mybir.AluOpType.bypass,
                    replica_groups=[[0, 1, 2, 3], [4, 5, 6, 7]],
                    ins=[kv_part[:]], outs=[kv_gath[:]])
                # gather-back on the gpsimd (SWDGE) ring: its sem wait on
                # the collective must not block the sync ring, and the next
                # gpsimd work (rowsum adds) isn't needed until attention.
                # Merged into 2 strided DMAs (Q7 desc-gen is ~1us per DMA).
                # tile_wait_until tells the TILE SCHEDULER (which re-orders
                # the whole program by simulated readiness) that kv lands
                # late (~115us: collective payload + peer skew) — without
                # it the scheduler hoists kv-dependent attention ahead of
                # ready prologue work and the in-order PE parks on it.
                kT_src = kv_gath[:].rearrange(
                    "(q x p) (a b) -> x p q a b", q=4, x=2, a=3)[0]
                v_src = kv_gath[:].rearrange(
                    "(q x p) (a b) -> x p a q b", q=4, x=2, a=4)[1]
                with tc.tile_wait_until(0.115):
                    nc.gpsimd.dma_start(
                        out=kT_sb[:].rearrange("p (q a) m -> p q a m", q=4),
                        in_=kT_src)
                    nc.gpsimd.dma_start(
                        out=v_sb[:].rearrange("p a (q b) -> p a q b", q=4),
                        in_=v_src)

                # ------- q pipeline pieces --------------------------------
                def qpipe_proj(ib):
                    """q projection + RMS-norm scale for 128-row block ib.
                    Returns the scaled bf16 q tile (natural layout)."""
                    if ib < len(xTb_pre):
                        xTb = xTb_pre[ib]
                    else:
                        xTb = workp.tile([128, DC, 128], BF16, name="xTb",
                                         tag="t3", bufs=8)
                        nc.sync.dma_start(
                            out=xTb[:],
                            in_=xT_r[:, :, ib * 128:(ib + 1) * 128])
                    qbf = workp.tile([128, H, 128], BF16, name="qbf",
                                     tag="t3", bufs=8)
                    ssq = workp.tile([128, H], F32, name="ssq", tag="tiny",
                                     bufs=12)
                    for ec in range(3):
                        psq = psp.tile([128, 512], F32, name="psq",
                                       tag="proj", bufs=2)
                        for c in range(DC):
                            nc.tensor.matmul(
                                psq[:], lhsT=xTb[:, c, :],
                                rhs=wq_sb[:, c, ec * 512:(ec + 1) * 512],
                                start=(c == 0), stop=(c == DC - 1))
                        scr = workp.tile([128, 512], F32, name="scr",
                                         tag="scr", bufs=2)
                        nc.scalar.copy(
                            qbf[:, ec * 4:(ec + 1) * 4, :],
                            psq[:].rearrange("p (a b) -> p a b", a=4))
                        nc.vector.tensor_mul(
                            scr[:].rearrange("p (a b) -> p a b", a=4),
                            qbf[:, ec * 4:(ec + 1) * 4, :],
                            qbf[:, ec * 4:(ec + 1) * 4, :])
                        nc.vector.tensor_reduce(
                            out=ssq[:, ec * 4:(ec + 1) * 4],
                            in_=scr[:].rearrange("p (a b) -> p a b", a=4),
                            axis=mybir.AxisListType.X, op=ADD)
                    # c = rsqrt(ssq + HD*eps), quake + 2 newton steps (DVE)
                    sse = workp.tile([128, H], F32, name="sse", tag="tiny",
                                     bufs=12)
                    nc.vector.tensor_scalar_add(sse[:], ssq[:], EPSH)
                    yi = workp.tile([128, H], I32, name="yi", tag="tiny",
                                    bufs=12)
                    nc.vector.tensor_scalar(
                        yi[:], sse[:].bitcast(I32), 1, -1,
                        op0=LSR, op1=XOR)
                    nc.vector.tensor_scalar_add(yi[:], yi[:], MAGIC_P1)
                    y = yi[:].bitcast(F32)
                    na = workp.tile([128, H], F32, name="na", tag="tiny",
                                    bufs=12)
                    for _ in range(2):
                        nc.vector.tensor_mul(na[:], sse[:], y)
                        nc.vector.tensor_mul(na[:], na[:], y)
                        nc.vector.tensor_scalar(
                            na[:], na[:], -0.5, 1.5, op0=MULT, op1=ADD)
                        nc.vector.tensor_mul(y, y, na[:])
                    for h in range(H):
                        nc.vector.tensor_scalar_mul(
                            qbf[:, h, :], qbf[:, h, :],
                            yi[:, h:h + 1].bitcast(F32))
                    return qbf

                def qpipe_trans(qbf, bi, qT):
                    """transpose a scaled q tile into qT[:, :, bi-block]."""
                    for t3c in range(3):
                        tps = psp.tile([128, 512], BF16, name="tps",
                                       tag="proj", bufs=2)
                        for cc in range(4):
                            h = t3c * 4 + cc
                            nc.tensor.transpose(
                                tps[:, cc * 128:(cc + 1) * 128],
                                qbf[:, h, :], ident_b)
                        nc.scalar.copy(
                            qT[:, t3c * 4:(t3c + 1) * 4,
                               bi * 128:(bi + 1) * 128],
                            tps[:].rearrange("p (a b) -> p a b", a=4))

                qTs = {}
                oTs = {}
                pend_tr = []                 # (qbf, block)
                proj_left = list(range(NBLK))

                def gi_of(b):
                    return 0 if b < GROUPS[1][0] else (
                        1 if b < GROUPS[2][0] else 2)

                def do_proj():
                    b = proj_left.pop(0)
                    gi = gi_of(b)
                    if b == GROUPS[gi][0]:
                        qTs[gi] = workp.tile([128, H, 512], BF16, name="qT",
                                             tag="t12", bufs=3)
                    pend_tr.append((qpipe_proj(b), b))

                def do_trans():
                    qbf, b = pend_tr.pop(0)
                    gi = gi_of(b)
                    qpipe_trans(qbf, b - GROUPS[gi][0], qTs[gi])

                # ------- prologue: PRO blocks, transposes trail by one ----
                do_proj()
                do_proj()
                for _ in range(PRO - 2):
                    do_trans()
                    do_proj()
                do_trans()
                do_trans()

                # ------- attention, software-pipelined one head deep ------
                def attn_scores(h, gw, qT):
                    """scores + exp for head h; rowsum partials on gpsimd/
                    DVE. Returns (pT, rsum)."""
                    pT = workp.tile([128, JB, 512], BF16, name="pT",
                                    tag="pT", bufs=2)
                    prt = workp.tile([128, 2, 512], BF16, name="prt",
                                     tag="prs", bufs=2)
                    for half in range(2):
                        sT = psp.tile([128, 2, 512], F32, name="sT",
                                      tag="sT", bufs=2)
                        for jj in range(2):
                            jb = half * 2 + jj
                            nc.tensor.matmul(
                                sT[:, jj, :gw],
                                lhsT=kT_sb[:, h, jb * 128:(jb + 1) * 128],
                                rhs=qT[:, h, :gw], start=True, stop=True)
                        nc.scalar.activation(
                            pT[:, half * 2:(half + 1) * 2, :gw],
                            sT[:, :, :gw], EXP)
                        nc.vector.tensor_add(
                            prt[:, half, :gw],
                            pT[:, half * 2, :gw],
                            pT[:, half * 2 + 1, :gw])
                    rsum = workp.tile([128, 512], BF16, name="rsum",
                                      tag="prs2", bufs=2)
                    nc.vector.tensor_add(
                        rsum[:, :gw], prt[:, 0, :gw], prt[:, 1, :gw])
                    return pT, rsum

                def attn_tail(h, gw, pT, rsum, oT):
                    """AV + single rowsum matmul + normalize for head h."""
                    av = psp.tile([128, 512], F32, name="av", tag="acc",
                                  bufs=2)
                    for jb in range(JB):
                        nc.tensor.matmul(
                            av[:, :gw],
                            lhsT=v_sb[:, jb, h * 128:(h + 1) * 128],
                            rhs=pT[:, jb, :gw],
                            start=(jb == 0), stop=(jb == JB - 1))
                    sm = psp.tile([128, 512], F32, name="sm", tag="acc",
                                  bufs=2)
                    nc.tensor.matmul(
                        sm[:, :gw], lhsT=ones_b[:], rhs=rsum[:, :gw],
                        start=True, stop=True)
                    rs = workp.tile([128, 512], F32, name="rs", tag="s2",
                                    bufs=4)
                    nc.vector.reciprocal_approx_fast(rs[:, :gw], sm[:, :gw])
                    nc.vector.tensor_mul(
                        oT[:, h, :gw], av[:, :gw], rs[:, :gw])

                def outproj_chunk(ib, bi, ec, oT):
                    sl = slice(ec * 512, (ec + 1) * 512)
                    po = psp.tile([128, 512], F32, name="po", tag="proj",
                                  bufs=2)
                    for hh in range(H):
                        nc.tensor.matmul(
                            po[:],
                            lhsT=oT[:, hh, bi * 128:(bi + 1) * 128],
                            rhs=wo_sb[:, hh, sl],
                            start=(hh == 0), stop=(hh == H - 1))
                    och = workp.tile([128, 512], F32, name="och", tag="s2",
                                     bufs=4)
                    nc.vector.tensor_copy(och[:], po[:])
                    nc.sync.dma_start(
                        out=out_d[ib * 128:(ib + 1) * 128, sl], in_=och[:])

                pend = None
                for gi, (g0b, gn) in enumerate(GROUPS):
                    gw = gn * 128
                    oTs[gi] = workp.tile([128, H, 512], BF16, name="oT",
                                         tag="oT", bufs=2)
                    op_left = []
                    if gi > 0:
                        pg0, pgn = GROUPS[gi - 1]
                        op_left = [(pg0 + bi, bi, ec)
                                   for bi in range(pgn) for ec in range(3)]
                    for h in range(H):
                        if h == 0 and pend is not None:
                            # finish prev group before its outproj fillers
                            attn_tail(*pend)
                            pend = None
                        cur = attn_scores(h, gw, qTs[gi])
                        # PE filler between scores(h) and tail(h-1): covers
                        # the exp+rowsum latency of head h.  Skipped at h==0
                        # so the first chunk never waits on the just-flushed
                        # last head of the previous group's oT.
                        if op_left and h > 0:
                            ib, bi, ec = op_left.pop(0)
                            outproj_chunk(ib, bi, ec, oTs[gi - 1])
                        elif gi == 0:
                            if h % 2 == 1 and proj_left:
                                do_proj()
                            elif pend_tr:
                                do_trans()
                        if pend is not None:
                            attn_tail(*pend)
                        pend = (h, gw, cur[0], cur[1], oTs[gi])
                    while op_left:
                        ib, bi, ec = op_left.pop(0)
                        outproj_chunk(ib, bi, ec, oTs[gi - 1])
                    while pend_tr:
                        do_trans()
                attn_tail(*pend)
                # epilogue: outproj of last group
                lg0, lgn = GROUPS[-1]
                for bi in range(lgn):
                    for ec in range(3):
                        outproj_chunk(lg0 + bi, bi, ec, oTs[len(GROUPS) - 1])

            if reps == 1:
                body()
            else:
                with tc.For_i(0, reps, 1):
                    body()
    nc.finalize()
    return nc


def kernel(x, context, wq, bq, wkv, bkv, wo, bo, q_norm_scale):
    x = np.asarray(x, dtype=np.float32)
    context = np.asarray(context, dtype=np.float32)
    bf = ml_dtypes.bfloat16

    if "nc" not in _cache:
        _cache["nc"] = _build()
    nc = _cache["nc"]

    scale_t = np.tile(np.asarray(q_norm_scale, np.float32), H)      # [D]
    wkv_f = np.asarray(wkv, np.float32)
    wk_b = (wkv_f[:, :D] * scale_t[None, :]).astype(bf)
    wv_b = np.ascontiguousarray(wkv_f[:, D:]).astype(bf)
    wq_b = np.asarray(wq, np.float32).astype(bf)
    wo_b = np.asarray(wo, np.float32).astype(bf)

    xp = np.zeros((B, CPB * RPC, D), np.float32)
    xp[:, :N] = x
    ctxT_b = [np.ascontiguousarray(context[b].T).astype(bf) for b in range(B)]

    in_maps = []
    for core in range(NCORES):
        b, q = divmod(core, CPB)
        xT = np.ascontiguousarray(xp[b, q * RPC:(q + 1) * RPC].T).astype(bf)
        in_maps.append({
            "xT": xT,
            "ctxT": ctxT_b[b],
            "wq": wq_b,
            "wkp": np.ascontiguousarray(wk_b[:, q * 384:(q + 1) * 384]),
            "wvp": np.ascontiguousarray(wv_b[:, q * 384:(q + 1) * 384]),
            "wo": wo_b,
        })

    res = bass_utils.run_bass_kernel_spmd(
        nc, in_maps, core_ids=list(range(NCORES)), trace=TRACE)
    _cache["last_results"] = res

    out = np.empty((B, N, D), np.float32)
    for b in range(B):
        cat = np.concatenate(
            [res.results[b * CPB + q]["out"] for q in range(CPB)], axis=0)
        out[b] = cat[:N]
    return out


# revision 23
# speedup vs baseline: 1.1630x; 1.0223x over previous
"""Cross-attention (q-norm variant) Trainium2 Bass kernel, v4.

Sharding: batch (2) x row-quarters (4) -> 8 cores, data-parallel over the
query sequence. Each core handles 1408 query rows (5376 padded to 5632 per
batch) of ONE batch, with that batch's context replicated. kv projection is
computed 1/4-per-core within each batch group and all-gathered.

v4 changes over v2 (all scheduling / engine-balance; math identical):
  - DMA issue is sequencer-FIFO-ordered with ~0.6-1us issue cost and
    head-of-line sem blocking, so DMAs are laid out per-ring by priority:
    sync ring gets the kv-feeding loads (coarse chunks), then wq/wo/x
    prefetches, then kv_part export (whose sem wait parks the ring at a
    point where nothing else is pending); the collective gather-back DMAs
    live on the gpsimd SWDGE ring whose next work (rowsum adds) isn't
    needed until attention anyway.  The collective now dispatches at
    ~27us instead of ~64us.
  - softmax row sums: the 4 ones-matmuls per head are replaced by
    gpsimd/DVE partial sums over the 4 j-blocks (pT0+pT1, pT2+pT3 on
    gpsimd, final add on DVE) + ONE ones-matmul -> 3/4 of that PE time
    moves to otherwise-idle engines.
  - attention is software-pipelined one head deep: head h's scores+exp are
    emitted before head h-1's AV/rowsum/normalize, so the exp latency is
    covered by real PE work instead of stalling.
  - AV matmuls run before the rowsum matmul (rowsum partials are still in
    flight on gpsimd while AV streams).
  - prologue projects PRO=7 q row-blocks before attention group 0 so the
    PE stays busy through the kv all-gather (~35us payload transfer).
  - outproj PSUM->SBUF copies moved from ACT to DVE (ACT keeps only exp
    and q-proj copies).

Host-side prep (numpy): cast weights to bf16, fold q_norm_scale into the
k-half of wkv, transpose x and context. Biases are structurally zero in
this problem and are dropped.
"""

import sys
import numpy as np

for _p in ("/opt/trn_rl_repo",):
    if _p not in sys.path:
        sys.path.insert(0, _p)

import ml_dtypes
import concourse.bass as bass
import concourse.tile as tile
from concourse import bacc, mybir
from concourse import bass_utils
from concourse.masks import make_identity

F32 = mybir.dt.float32
BF16 = mybir.dt.bfloat16
I32 = mybir.dt.int32
EXP = mybir.ActivationFunctionType.Exp
MULT = mybir.AluOpType.mult
ADD = mybir.AluOpType.add
LSR = mybir.AluOpType.logical_shift_right
XOR = mybir.AluOpType.bitwise_xor

B, N, D, M, H, HD = 2, 5376, 1536, 512, 12, 128
EPS = 1e-6
EPSH = float(HD * EPS)
NCORES = 8
CPB = 4            # cores per batch
RPC = 1408         # padded rows per core  (4*1408 = 5632 >= 5376)
NBLK = RPC // 128  # 11
DC = D // 128      # 12 contraction chunks
JB = M // 128      # 4 context row blocks
GROUPS = [(0, 4), (4, 4), (8, 3)]   # (start block, #blocks)
PRO = 11           # q row-blocks projected before attention starts
                   # (all of them: covers the collective's peer-skew tail)
MAGIC_P1 = 0x5F3759E0  # quake rsqrt magic + 1

TRACE = False

_cache = {}


def _build(reps=1):
    nc = bacc.Bacc(
        "TRN2", target_bir_lowering=False, debug=False, num_devices=NCORES
    )
    xT_d = nc.dram_tensor("xT", [D, RPC], BF16, kind="ExternalInput").ap()
    ctxT_d = nc.dram_tensor("ctxT", [D, M], BF16, kind="ExternalInput").ap()
    wq_d = nc.dram_tensor("wq", [D, D], BF16, kind="ExternalInput").ap()
    # per-core slices: 3 k-heads / 384 v-cols per core; kv is all-gathered
    wkp_d = nc.dram_tensor("wkp", [D, 384], BF16, kind="ExternalInput").ap()
    wvp_d = nc.dram_tensor("wvp", [D, 384], BF16, kind="ExternalInput").ap()
    wo_d = nc.dram_tensor("wo", [D, D], BF16, kind="ExternalInput").ap()
    out_d = nc.dram_tensor("out", [RPC, D], F32, kind="ExternalOutput").ap()

    xT_r = xT_d.rearrange("(c p) n -> p c n", p=128)      # [128, DC, RPC]
    ctxT_r = ctxT_d.rearrange("(c p) m -> p c m", p=128)  # [128, DC, M]
    wq_r = wq_d.rearrange("(c p) n -> p c n", p=128)
    wkp_r = wkp_d.rearrange("(c p) n -> p c n", p=128)
    wvp_r = wvp_d.rearrange("(c p) n -> p c n", p=128)
    wo_r = wo_d.rearrange("(c p) n -> p c n", p=128)

    with tile.TileContext(nc) as tc:
        with (
            tc.tile_pool(name="const", bufs=1) as constp,
            tc.tile_pool(name="wts", bufs=1) as wtp,
            tc.tile_pool(name="work", bufs=2) as workp,
            tc.tile_pool(name="dram", bufs=1, space="DRAM") as dramp,
            tc.tile_pool(name="ps", bufs=2, space="PSUM") as psp,
        ):
            ident_b = constp.tile([128, 128], BF16, name="ident_b")
            make_identity(nc, ident_b)
            ones_b = constp.tile([128, 128], BF16, name="ones_b")
            nc.vector.memset(ones_b[:], 1.0)

            wq_sb = wtp.tile([128, DC, D], BF16, name="wq_sb")
            wo_sb = wtp.tile([128, DC, D], BF16, name="wo_sb")
            kT_sb = wtp.tile([128, H, M], BF16, name="kT_sb")   # [d, h, j]
            v_sb = wtp.tile([128, JB, D], BF16, name="v_sb")    # [j, jb, hd]

            def body():
                # ------- phase A DMAs ------------------------------------
                # Every dma_start occupies its issuing sequencer ~0.6-1us in
                # strict FIFO order, and a DMA whose input isn't ready blocks
                # the whole ring behind it.  So: kv-feeding DMAs first on the
                # sync ring (coarse chunks, ~10 issues), wq/wo/x-prefetch
                # next (no input deps), kv_part export last (its sem wait
                # parks the ring until ~25us, when nothing else is pending).
                ctxT_sb = workp.tile([128, DC, M], BF16, name="ctxT_sb",
                                     tag="t12", bufs=3)
                wkp_sb = workp.tile([128, DC, 384], BF16, name="wkp_sb",
                                    tag="t12", bufs=3)
                wvp_sb = workp.tile([128, DC, 384], BF16, name="wvp_sb",
                                    tag="t12", bufs=3)
                # k-feeds first (k chains are the first PE work), then v,
                # then x-prefetch, then wq in COLUMN slices (so qproj ec=0
                # can start before all of wq lands), then wo.  HBM is the
                # scarce resource 0-35us: everything here is ordered by
                # first-use time.
                kv_pieces = [slice(0, 1), slice(1, 3), slice(3, 6),
                             slice(6, 9), slice(9, 12)]
                for c3 in kv_pieces:
                    nc.sync.dma_start(out=wkp_sb[:, c3, :],
                                      in_=wkp_r[:, c3, :])
                    nc.sync.dma_start(out=ctxT_sb[:, c3, :],
                                      in_=ctxT_r[:, c3, :])
                nc.sync.dma_start(out=wq_sb[:, :, 0:512],
                                  in_=wq_r[:, :, 0:512])
                nc.sync.dma_start(out=wvp_sb[:, 0:6, :], in_=wvp_r[:, 0:6, :])
                nc.sync.dma_start(out=wvp_sb[:, 6:12, :],
                                  in_=wvp_r[:, 6:12, :])
                xTb_pre = []
                for ib in range(2):
                    xTb = workp.tile([128, DC, 128], BF16, name="xTb",
                                     tag="t3", bufs=8)
                    nc.sync.dma_start(
                        out=xTb[:], in_=xT_r[:, :, ib * 128:(ib + 1) * 128])
                    xTb_pre.append(xTb)
                nc.sync.dma_start(out=wq_sb[:, :, 512:1024],
                                  in_=wq_r[:, :, 512:1024])
                nc.sync.dma_start(out=wq_sb[:, :, 1024:1536],
                                  in_=wq_r[:, :, 1024:1536])

                # ------- kv projection partials (this core's 1/4) ---------
                kT_part = workp.tile([128, 3, M], BF16, name="kT_part",
                                     tag="t3", bufs=8)
                for h in range(3):
                    kps = psp.tile([128, M], F32, name="kps", tag="acc",
                                   bufs=2)
                    for c in range(DC):
                        nc.tensor.matmul(
                            kps[:],
                            lhsT=wkp_sb[:, c, h * 128:(h + 1) * 128],
                            rhs=ctxT_sb[:, c, :],
                            start=(c == 0), stop=(c == DC - 1))
                    nc.scalar.copy(kT_part[:, h, :], kps[:])
                v_part = workp.tile([128, JB, 384], BF16, name="v_part",
                                    tag="t3", bufs=8)
                for jb in range(JB):
                    vps = psp.tile([128, 512], F32, name="vps", tag="acc",
                                   bufs=2)
                    for c in range(DC):
                        nc.tensor.matmul(
                            vps[:, :384],
                            lhsT=ctxT_sb[:, c, jb * 128:(jb + 1) * 128],
                            rhs=wvp_sb[:, c, :],
                            start=(c == 0), stop=(c == DC - 1))
                    nc.scalar.copy(v_part[:, jb, :], vps[:, :384])

                kv_part = dramp.tile([256, 1536], BF16, name="kv_part")
                nc.sync.dma_start(out=kv_part[0:128, :], in_=kT_part[:])
                nc.sync.dma_start(out=kv_part[128:256, :], in_=v_part[:])
                # wo AFTER kv_part: a huge in-flight DMA ahead of kv_part on
                # the shared completion-sem lanes would delay the collective
                nc.sync.dma_start(out=wo_sb[:], in_=wo_r)
                kv_gath = dramp.tile([1024, 1536], BF16, name="kv_gath")
                nc.gpsimd.collective_compute(
                    "AllGather", mybir.AluOpType.bypass,
                    replica_groups=[[0, 1, 2, 3], [4, 5, 6, 7]],
                    ins=[kv_part[:]], outs=[kv_gath[:]])
                # gather-back on the gpsimd (SWDGE) ring: its sem wait on
                # the collective must not block the sync ring, and the next
                # gpsimd work (rowsum adds) isn't needed until attention.
                # Merged into 2 strided DMAs (Q7 desc-gen is ~1us per DMA).
                # tile_wait_until tells the TILE SCHEDULER (which re-orders
                # the whole program by simulated readiness) that kv lands
                # late (~115us: collective payload + peer skew) — without
                # it the scheduler hoists kv-dependent attention ahead of
                # ready prologue work and the in-order PE parks on it.
                kT_src = kv_gath[:].rearrange(
                    "(q x p) (a b) -> x p q a b", q=4, x=2, a=3)[0]
                v_src = kv_gath[:].rearrange(
                    "(q x p) (a b) -> x p a q b", q=4, x=2, a=4)[1]
                with tc.tile_wait_until(0.115):
                    nc.gpsimd.dma_start(
                        out=kT_sb[:].rearrange("p (q a) m -> p q a m", q=4),
                        in_=kT_src)
                    nc.gpsimd.dma_start(
                        out=v_sb[:].rearrange("p a (q b) -> p a q b", q=4),
                        in_=v_src)

                # ------- q pipeline pieces --------------------------------
                def qpipe_proj(ib):
                    """q projection + RMS-norm scale for 128-row block ib.
                    Returns the scaled bf16 q tile (natural layout)."""
                    if ib < len(xTb_pre):
                        xTb = xTb_pre[ib]
                    else:
                        xTb = workp.tile([128, DC, 128], BF16, name="xTb",
                                         tag="t3", bufs=8)
                        nc.sync.dma_start(
                            out=xTb[:],
                            in_=xT_r[:, :, ib * 128:(ib + 1) * 128])
                    qbf = workp.tile([128, H, 128], BF16, name="qbf",
                                     tag="t3", bufs=8)
                    ssq = workp.tile([128, H], F32, name="ssq", tag="tiny",
                                     bufs=12)
                    for ec in range(3):
                        psq = psp.tile([128, 512], F32, name="psq",
                                       tag="proj", bufs=2)
                        for c in range(DC):
                            nc.tensor.matmul(
                                psq[:], lhsT=xTb[:, c, :],
                                rhs=wq_sb[:, c, ec * 512:(ec + 1) * 512],
                                start=(c == 0), stop=(c == DC - 1))
                        scr = workp.tile([128, 512], F32, name="scr",
                                         tag="scr", bufs=2)
                        nc.scalar.copy(
                            qbf[:, ec * 4:(ec + 1) * 4, :],
                            psq[:].rearrange("p (a b) -> p a b", a=4))
                        nc.vector.tensor_mul(
                            scr[:].rearrange("p (a b) -> p a b", a=4),
                            qbf[:, ec * 4:(ec + 1) * 4, :],
                            qbf[:, ec * 4:(ec + 1) * 4, :])
                        nc.vector.tensor_reduce(
                            out=ssq[:, ec * 4:(ec + 1) * 4],
                            in_=scr[:].rearrange("p (a b) -> p a b", a=4),
                            axis=mybir.AxisListType.X, op=ADD)
                    # c = rsqrt(ssq + HD*eps), quake + 2 newton steps (DVE)
                    sse = workp.tile([128, H], F32, name="sse", tag="tiny",
                                     bufs=12)
                    nc.vector.tensor_scalar_add(sse[:], ssq[:], EPSH)
                    yi = workp.tile([128, H], I32, name="yi", tag="tiny",
                                    bufs=12)
                    nc.vector.tensor_scalar(
                        yi[:], sse[:].bitcast(I32), 1, -1,
                        op0=LSR, op1=XOR)
                    nc.vector.tensor_scalar_add(yi[:], yi[:], MAGIC_P1)
                    y = yi[:].bitcast(F32)
                    na = workp.tile([128, H], F32, name="na", tag="tiny",
                                    bufs=12)
                    for _ in range(2):
                        nc.vector.tensor_mul(na[:], sse[:], y)
                        nc.vector.tensor_mul(na[:], na[:], y)
                        nc.vector.tensor_scalar(
                            na[:], na[:], -0.5, 1.5, op0=MULT, op1=ADD)
                        nc.vector.tensor_mul(y, y, na[:])
                    for h in range(H):
                        nc.vector.tensor_scalar_mul(
                            qbf[:, h, :], qbf[:, h, :],
                            yi[:, h:h + 1].bitcast(F32))
                    return qbf

                def qpipe_trans(qbf, bi, qT):
                    """transpose a scaled q tile into qT[:, :, bi-block]."""
                    for t3c in range(3):
                        # "acc" tag: av/sm are idle during the prologue
                        # (where ALL transposes now run), while the "proj"
                        # slots stay dedicated to the psq chains.
                        tps = psp.tile([128, 512], BF16, name="tps",
                                       tag="acc", bufs=2)
                        for cc in range(4):
                            h = t3c * 4 + cc
                            nc.tensor.transpose(
                                tps[:, cc * 128:(cc + 1) * 128],
                                qbf[:, h, :], ident_b)
                        nc.scalar.copy(
                            qT[:, t3c * 4:(t3c + 1) * 4,
                               bi * 128:(bi + 1) * 128],
                            tps[:].rearrange("p (a b) -> p a b", a=4))

                qTs = {}
                oTs = {}
                pend_tr = []                 # (qbf, block)
                proj_left = list(range(NBLK))

                def gi_of(b):
                    return 0 if b < GROUPS[1][0] else (
                        1 if b < GROUPS[2][0] else 2)

                def do_proj():
                    b = proj_left.pop(0)
                    gi = gi_of(b)
                    if b == GROUPS[gi][0]:
                        qTs[gi] = workp.tile([128, H, 512], BF16, name="qT",
                                             tag="t12", bufs=3)
                    pend_tr.append((qpipe_proj(b), b))

                def do_trans():
                    qbf, b = pend_tr.pop(0)
                    gi = gi_of(b)
                    qpipe_trans(qbf, b - GROUPS[gi][0], qTs[gi])

                # ------- prologue: PRO blocks, transposes trail by one ----
                do_proj()
                do_proj()
                for _ in range(PRO - 2):
                    do_trans()
                    do_proj()
                do_trans()
                do_trans()

                # ------- attention, software-pipelined one head deep ------
                def attn_scores(h, gw, qT):
                    """scores + exp for head h; rowsum partials on gpsimd/
                    DVE. Returns (pT, rsum)."""
                    pT = workp.tile([128, JB, 512], BF16, name="pT",
                                    tag="pT", bufs=2)
                    prt = workp.tile([128, 2, 512], BF16, name="prt",
                                     tag="prs", bufs=2)
                    for half in range(2):
                        sT = psp.tile([128, 2, 512], F32, name="sT",
                                      tag="sT", bufs=2)
                        for jj in range(2):
                            jb = half * 2 + jj
                            nc.tensor.matmul(
                                sT[:, jj, :gw],
                                lhsT=kT_sb[:, h, jb * 128:(jb + 1) * 128],
                                rhs=qT[:, h, :gw], start=True, stop=True)
                        nc.scalar.activation(
                            pT[:, half * 2:(half + 1) * 2, :gw],
                            sT[:, :, :gw], EXP)
                        nc.vector.tensor_add(
                            prt[:, half, :gw],
                            pT[:, half * 2, :gw],
                            pT[:, half * 2 + 1, :gw])
                    rsum = workp.tile([128, 512], BF16, name="rsum",
                                      tag="prs2", bufs=2)
                    nc.vector.tensor_add(
                        rsum[:, :gw], prt[:, 0, :gw], prt[:, 1, :gw])
                    return pT, rsum

                def attn_tail(h, gw, pT, rsum, oT):
                    """AV + single rowsum matmul + normalize for head h."""
                    av = psp.tile([128, 512], F32, name="av", tag="acc",
                                  bufs=2)
                    for jb in range(JB):
                        nc.tensor.matmul(
                            av[:, :gw],
                            lhsT=v_sb[:, jb, h * 128:(h + 1) * 128],
                            rhs=pT[:, jb, :gw],
                            start=(jb == 0), stop=(jb == JB - 1))
                    sm = psp.tile([128, 512], F32, name="sm", tag="acc",
                                  bufs=2)
                    nc.tensor.matmul(
                        sm[:, :gw], lhsT=ones_b[:], rhs=rsum[:, :gw],
                        start=True, stop=True)
                    rs = workp.tile([128, 512], F32, name="rs", tag="s2",
                                    bufs=4)
                    nc.vector.reciprocal_approx_fast(rs[:, :gw], sm[:, :gw])
                    nc.vector.tensor_mul(
                        oT[:, h, :gw], av[:, :gw], rs[:, :gw])

                def outproj_chunk(ib, bi, ec, oT):
                    sl = slice(ec * 512, (ec + 1) * 512)
                    po = psp.tile([128, 512], F32, name="po", tag="proj",
                                  bufs=2)
                    for hh in range(H):
                        nc.tensor.matmul(
                            po[:],
                            lhsT=oT[:, hh, bi * 128:(bi + 1) * 128],
                            rhs=wo_sb[:, hh, sl],
                            start=(hh == 0), stop=(hh == H - 1))
                    och = workp.tile([128, 512], F32, name="och", tag="s2",
                                     bufs=4)
                    nc.vector.tensor_copy(och[:], po[:])
                    nc.sync.dma_start(
                        out=out_d[ib * 128:(ib + 1) * 128, sl], in_=och[:])

                pend = None
                for gi, (g0b, gn) in enumerate(GROUPS):
                    gw = gn * 128
                    oTs[gi] = workp.tile([128, H, 512], BF16, name="oT",
                                         tag="oT", bufs=2)
                    op_left = []
                    if gi > 0:
                        pg0, pgn = GROUPS[gi - 1]
                        op_left = [(pg0 + bi, bi, ec)
                                   for bi in range(pgn) for ec in range(3)]
                    for h in range(H):
                        if h == 0 and pend is not None:
                            # finish prev group before its outproj fillers
                            attn_tail(*pend)
                            pend = None
                        cur = attn_scores(h, gw, qTs[gi])
                        # PE filler between scores(h) and tail(h-1): covers
                        # the exp+rowsum latency of head h.  Skipped at h==0
                        # so the first chunk never waits on the just-flushed
                        # last head of the previous group's oT.
                        if op_left and h > 0:
                            ib, bi, ec = op_left.pop(0)
                            outproj_chunk(ib, bi, ec, oTs[gi - 1])
                        elif gi == 0:
                            if h % 2 == 1 and proj_left:
                                do_proj()
                            elif pend_tr:
                                do_trans()
                        if pend is not None:
                            attn_tail(*pend)
                        pend = (h, gw, cur[0], cur[1], oTs[gi])
                    while op_left:
                        ib, bi, ec = op_left.pop(0)
                        outproj_chunk(ib, bi, ec, oTs[gi - 1])
                    while pend_tr:
                        do_trans()
                attn_tail(*pend)
                # epilogue: outproj of last group
                lg0, lgn = GROUPS[-1]
                for bi in range(lgn):
                    for ec in range(3):
                        outproj_chunk(lg0 + bi, bi, ec, oTs[len(GROUPS) - 1])

            if reps == 1:
                body()
            else:
                with tc.For_i(0, reps, 1):
                    body()
    nc.finalize()
    return nc


def kernel(x, context, wq, bq, wkv, bkv, wo, bo, q_norm_scale):
    x = np.asarray(x, dtype=np.float32)
    context = np.asarray(context, dtype=np.float32)
    bf = ml_dtypes.bfloat16

    if "nc" not in _cache:
        _cache["nc"] = _build()
    nc = _cache["nc"]

    scale_t = np.tile(np.asarray(q_norm_scale, np.float32), H)      # [D]
    wkv_f = np.asarray(wkv, np.float32)
    wk_b = (wkv_f[:, :D] * scale_t[None, :]).astype(bf)
    wv_b = np.ascontiguousarray(wkv_f[:, D:]).astype(bf)
    wq_b = np.asarray(wq, np.float32).astype(bf)
    wo_b = np.asarray(wo, np.float32).astype(bf)

    xp = np.zeros((B, CPB * RPC, D), np.float32)
    xp[:, :N] = x
    ctxT_b = [np.ascontiguousarray(context[b].T).astype(bf) for b in range(B)]

    in_maps = []
    for core in range(NCORES):
        b, q = divmod(core, CPB)
        xT = np.ascontiguousarray(xp[b, q * RPC:(q + 1) * RPC].T).astype(bf)
        in_maps.append({
            "xT": xT,
            "ctxT": ctxT_b[b],
            "wq": wq_b,
            "wkp": np.ascontiguousarray(wk_b[:, q * 384:(q + 1) * 384]),
            "wvp": np.ascontiguousarray(wv_b[:, q * 384:(q + 1) * 384]),
            "wo": wo_b,
        })

    res = bass_utils.run_bass_kernel_spmd(
        nc, in_maps, core_ids=list(range(NCORES)), trace=TRACE)
    _cache["last_results"] = res

    out = np.empty((B, N, D), np.float32)
    for b in range(B):
        cat = np.concatenate(
            [res.results[b * CPB + q]["out"] for q in range(CPB)], axis=0)
        out[b] = cat[:N]
    return out


# revision 26
# speedup vs baseline: 1.1902x; 1.0234x over previous
"""Cross-attention (q-norm variant) Trainium2 Bass kernel, v9 (356.9us).

Sharding: batch (2) x row-quarters (4) -> 8 cores, data-parallel over the
query sequence. Each core handles 1408 query rows (5376 padded to 5632 per
batch) of ONE batch, with that batch's context replicated. kv projection is
computed 1/4-per-core within each batch group and all-gathered.

Evolution v2 (452us) -> v9 (357us), all scheduling/engine-balance, math
identical (rel err 5.9e-3 throughout):
  - softmax row sums: the 4 ones-matmuls per head are replaced by DVE
    partial adds over the 4 j-blocks + ONE ones-matmul -> 3/4 of that PE
    time moves off the critical engine.  (gpsimd tensor_add was tried and
    reverted: Pool-engine adds run at 0.42 efficiency, ~1.1us each, and
    stall the per-head chain.)
  - attention is software-pipelined one head deep: head h's scores+exp
    are emitted before head h-1's AV/rowsum/normalize; AV runs before the
    rowsum matmul so the partial sums are still in flight while AV
    streams.  outproj chunks of group g-1 are the between-head PE fillers
    for group g (skipped at h==0 to not touch the just-flushed oT).
  - the ENTIRE q pipeline (all 11 row-blocks: projection, RMS-norm,
    transpose) runs before attention group 0.  This keeps the PE busy
    through the kv all-gather, whose payload wait has 27-49us run-to-run
    variance (peer skew).
  - KEY FIX: the Tile scheduler re-orders the program by simulated
    readiness and models the collective as fast, so it was hoisting the
    kv-dependent first scores matmul ahead of ~50us of ready prologue
    work, parking the in-order PE queue.  tc.tile_wait_until(0.115)
    around the gather-back DMAs tells the scheduler kv really lands
    ~115us in, which makes it place all prologue work first.
  - DMA layout: each dma_start costs ~0.6-1us on its issuing sequencer
    (FIFO, head-of-line blocking on unmet input sems).  The sync ring
    carries kv feeds (small first piece for a fast PE start), wq in
    column slices (qproj ec=0 can start early), x prefetches, and the
    kv_part export; the gather-back DMAs live on the gpsimd SWDGE ring
    (merged to 2 strided DMAs) since its next work isn't needed until
    attention; wo goes after kv_part so its completion doesn't delay the
    collective trigger through shared DMA-completion sem lanes.
  - transposes write PSUM tag "acc" (idle during the prologue where all
    transposes now run) instead of sharing "proj" with the psq chains,
    killing a 665ns-per-block rotation stall.
  - outproj PSUM->SBUF copies on DVE, qT copies on ACT (frees the tps
    PSUM slot fastest); exp stays on ACT (only engine with exp).

Host-side prep (numpy): cast weights to bf16, fold q_norm_scale into the
k-half of wkv, transpose x and context. Biases are structurally zero in
this problem and are dropped.
"""

import sys
import numpy as np

for _p in ("/opt/trn_rl_repo",):
    if _p not in sys.path:
        sys.path.insert(0, _p)

import ml_dtypes
import concourse.bass as bass
import concourse.tile as tile
from concourse import bacc, mybir
from concourse import bass_utils
from concourse.masks import make_identity

F32 = mybir.dt.float32
BF16 = mybir.dt.bfloat16
I32 = mybir.dt.int32
EXP = mybir.ActivationFunctionType.Exp
MULT = mybir.AluOpType.mult
ADD = mybir.AluOpType.add
LSR = mybir.AluOpType.logical_shift_right
XOR = mybir.AluOpType.bitwise_xor

B, N, D, M, H, HD = 2, 5376, 1536, 512, 12, 128
EPS = 1e-6
EPSH = float(HD * EPS)
NCORES = 8
CPB = 4            # cores per batch
RPC = 1408         # padded rows per core  (4*1408 = 5632 >= 5376)
NBLK = RPC // 128  # 11
DC = D // 128      # 12 contraction chunks
JB = M // 128      # 4 context row blocks
GROUPS = [(0, 4), (4, 4), (8, 3)]   # (start block, #blocks)
PRO = 11           # q row-blocks projected before attention starts
                   # (all of them: covers the collective's peer-skew tail)
MAGIC_P1 = 0x5F3759E0  # quake rsqrt magic + 1

TRACE = False

_cache = {}


def _build(reps=1):
    nc = bacc.Bacc(
        "TRN2", target_bir_lowering=False, debug=False, num_devices=NCORES
    )
    xT_d = nc.dram_tensor("xT", [D, RPC], BF16, kind="ExternalInput").ap()
    ctxT_d = nc.dram_tensor("ctxT", [D, M], BF16, kind="ExternalInput").ap()
    wq_d = nc.dram_tensor("wq", [D, D], BF16, kind="ExternalInput").ap()
    # per-core slices: 3 k-heads / 384 v-cols per core; kv is all-gathered
    wkp_d = nc.dram_tensor("wkp", [D, 384], BF16, kind="ExternalInput").ap()
    wvp_d = nc.dram_tensor("wvp", [D, 384], BF16, kind="ExternalInput").ap()
    wo_d = nc.dram_tensor("wo", [D, D], BF16, kind="ExternalInput").ap()
    out_d = nc.dram_tensor("out", [RPC, D], F32, kind="ExternalOutput").ap()

    xT_r = xT_d.rearrange("(c p) n -> p c n", p=128)      # [128, DC, RPC]
    ctxT_r = ctxT_d.rearrange("(c p) m -> p c m", p=128)  # [128, DC, M]
    wq_r = wq_d.rearrange("(c p) n -> p c n", p=128)
    wkp_r = wkp_d.rearrange("(c p) n -> p c n", p=128)
    wvp_r = wvp_d.rearrange("(c p) n -> p c n", p=128)
    wo_r = wo_d.rearrange("(c p) n -> p c n", p=128)

    with tile.TileContext(nc) as tc:
        with (
            tc.tile_pool(name="const", bufs=1) as constp,
            tc.tile_pool(name="wts", bufs=1) as wtp,
            tc.tile_pool(name="work", bufs=2) as workp,
            tc.tile_pool(name="dram", bufs=1, space="DRAM") as dramp,
            tc.tile_pool(name="ps", bufs=2, space="PSUM") as psp,
        ):
            ident_b = constp.tile([128, 128], BF16, name="ident_b")
            make_identity(nc, ident_b)
            ones_b = constp.tile([128, 128], BF16, name="ones_b")
            nc.vector.memset(ones_b[:], 1.0)

            wq_sb = wtp.tile([128, DC, D], BF16, name="wq_sb")
            wo_sb = wtp.tile([128, DC, D], BF16, name="wo_sb")
            kT_sb = wtp.tile([128, H, M], BF16, name="kT_sb")   # [d, h, j]
            v_sb = wtp.tile([128, JB, D], BF16, name="v_sb")    # [j, jb, hd]

            def body():
                # ------- phase A DMAs ------------------------------------
                # Every dma_start occupies its issuing sequencer ~0.6-1us in
                # strict FIFO order, and a DMA whose input isn't ready blocks
                # the whole ring behind it.  So: kv-feeding DMAs first on the
                # sync ring (coarse chunks, ~10 issues), wq/wo/x-prefetch
                # next (no input deps), kv_part export last (its sem wait
                # parks the ring until ~25us, when nothing else is pending).
                ctxT_sb = workp.tile([128, DC, M], BF16, name="ctxT_sb",
                                     tag="t12", bufs=3)
                wkp_sb = workp.tile([128, DC, 384], BF16, name="wkp_sb",
                                    tag="t12", bufs=3)
                wvp_sb = workp.tile([128, DC, 384], BF16, name="wvp_sb",
                                    tag="t12", bufs=3)
                # k-feeds first (k chains are the first PE work), then v,
                # then x-prefetch, then wq in COLUMN slices (so qproj ec=0
                # can start before all of wq lands), then wo.  HBM is the
                # scarce resource 0-35us: everything here is ordered by
                # first-use time.
                kv_pieces = [slice(0, 1), slice(1, 3), slice(3, 6),
                             slice(6, 9), slice(9, 12)]
                for c3 in kv_pieces:
                    nc.sync.dma_start(out=wkp_sb[:, c3, :],
                                      in_=wkp_r[:, c3, :])
                    nc.sync.dma_start(out=ctxT_sb[:, c3, :],
                                      in_=ctxT_r[:, c3, :])
                nc.sync.dma_start(out=wq_sb[:, :, 0:512],
                                  in_=wq_r[:, :, 0:512])
                nc.sync.dma_start(out=wvp_sb[:, 0:6, :], in_=wvp_r[:, 0:6, :])
                nc.sync.dma_start(out=wvp_sb[:, 6:12, :],
                                  in_=wvp_r[:, 6:12, :])
                xTb_pre = []
                for ib in range(2):
                    xTb = workp.tile([128, DC, 128], BF16, name="xTb",
                                     tag="t3", bufs=8)
                    nc.sync.dma_start(
                        out=xTb[:], in_=xT_r[:, :, ib * 128:(ib + 1) * 128])
                    xTb_pre.append(xTb)
                nc.sync.dma_start(out=wq_sb[:, :, 512:1024],
                                  in_=wq_r[:, :, 512:1024])
                nc.sync.dma_start(out=wq_sb[:, :, 1024:1536],
                                  in_=wq_r[:, :, 1024:1536])

                # ------- kv projection partials (this core's 1/4) ---------
                kT_part = workp.tile([128, 3, M], BF16, name="kT_part",
                                     tag="t3", bufs=8)
                for h in range(3):
                    kps = psp.tile([128, M], F32, name="kps", tag="acc",
                                   bufs=2)
                    for c in range(DC):
                        nc.tensor.matmul(
                            kps[:],
                            lhsT=wkp_sb[:, c, h * 128:(h + 1) * 128],
                            rhs=ctxT_sb[:, c, :],
                            start=(c == 0), stop=(c == DC - 1))
                    nc.scalar.copy(kT_part[:, h, :], kps[:])
                v_part = workp.tile([128, JB, 384], BF16, name="v_part",
                                    tag="t3", bufs=8)
                for jb in range(JB):
                    vps = psp.tile([128, 512], F32, name="vps", tag="acc",
                                   bufs=2)
                    for c in range(DC):
                        nc.tensor.matmul(
                            vps[:, :384],
                            lhsT=ctxT_sb[:, c, jb * 128:(jb + 1) * 128],
                            rhs=wvp_sb[:, c, :],
                            start=(c == 0), stop=(c == DC - 1))
                    nc.scalar.copy(v_part[:, jb, :], vps[:, :384])

                kv_part = dramp.tile([256, 1536], BF16, name="kv_part")
                nc.sync.dma_start(out=kv_part[0:128, :], in_=kT_part[:])
                nc.sync.dma_start(out=kv_part[128:256, :], in_=v_part[:])
                # wo AFTER kv_part: a huge in-flight DMA ahead of kv_part on
                # the shared completion-sem lanes would delay the collective
                nc.sync.dma_start(out=wo_sb[:], in_=wo_r)
                kv_gath = dramp.tile([1024, 1536], BF16, name="kv_gath")
                nc.gpsimd.collective_compute(
                    "AllGather", mybir.AluOpType.bypass,
                    replica_groups=[[0, 1, 2, 3], [4, 5, 6, 7]],
                    ins=[kv_part[:]], outs=[kv_gath[:]])
                # gather-back on the gpsimd (SWDGE) ring: its sem wait on
                # the collective must not block the sync ring, and the next
                # gpsimd work (rowsum adds) isn't needed until attention.
                # Merged into 2 strided DMAs (Q7 desc-gen is ~1us per DMA).
                # tile_wait_until tells the TILE SCHEDULER (which re-orders
                # the whole program by simulated readiness) that kv lands
                # late (~115us: collective payload + peer skew) — without
                # it the scheduler hoists kv-dependent attention ahead of
                # ready prologue work and the in-order PE parks on it.
                kT_src = kv_gath[:].rearrange(
                    "(q x p) (a b) -> x p q a b", q=4, x=2, a=3)[0]
                v_src = kv_gath[:].rearrange(
                    "(q x p) (a b) -> x p a q b", q=4, x=2, a=4)[1]
                with tc.tile_wait_until(0.115):
                    nc.gpsimd.dma_start(
                        out=kT_sb[:].rearrange("p (q a) m -> p q a m", q=4),
                        in_=kT_src)
                    nc.gpsimd.dma_start(
                        out=v_sb[:].rearrange("p a (q b) -> p a q b", q=4),
                        in_=v_src)

                # ------- q pipeline pieces --------------------------------
                def qpipe_proj(ib):
                    """q projection + RMS-norm scale for 128-row block ib.
                    Returns the scaled bf16 q tile (natural layout)."""
                    if ib < len(xTb_pre):
                        xTb = xTb_pre[ib]
                    else:
                        xTb = workp.tile([128, DC, 128], BF16, name="xTb",
                                         tag="t3", bufs=8)
                        nc.sync.dma_start(
                            out=xTb[:],
                            in_=xT_r[:, :, ib * 128:(ib + 1) * 128])
                    qbf = workp.tile([128, H, 128], BF16, name="qbf",
                                     tag="t3", bufs=8)
                    ssq = workp.tile([128, H], F32, name="ssq", tag="tiny",
                                     bufs=12)
                    for ec in range(3):
                        psq = psp.tile([128, 512], F32, name="psq",
                                       tag="proj", bufs=2)
                        for c in range(DC):
                            nc.tensor.matmul(
                                psq[:], lhsT=xTb[:, c, :],
                                rhs=wq_sb[:, c, ec * 512:(ec + 1) * 512],
                                start=(c == 0), stop=(c == DC - 1))
                        scr = workp.tile([128, 512], F32, name="scr",
                                         tag="scr", bufs=2)
                        nc.scalar.copy(
                            qbf[:, ec * 4:(ec + 1) * 4, :],
                            psq[:].rearrange("p (a b) -> p a b", a=4))
                        nc.vector.tensor_mul(
                            scr[:].rearrange("p (a b) -> p a b", a=4),
                            qbf[:, ec * 4:(ec + 1) * 4, :],
                            qbf[:, ec * 4:(ec + 1) * 4, :])
                        nc.vector.tensor_reduce(
                            out=ssq[:, ec * 4:(ec + 1) * 4],
                            in_=scr[:].rearrange("p (a b) -> p a b", a=4),
                            axis=mybir.AxisListType.X, op=ADD)
                    # c = rsqrt(ssq + HD*eps), quake + 2 newton steps (DVE)
                    sse = workp.tile([128, H], F32, name="sse", tag="tiny",
                                     bufs=12)
                    nc.vector.tensor_scalar_add(sse[:], ssq[:], EPSH)
                    yi = workp.tile([128, H], I32, name="yi", tag="tiny",
                                    bufs=12)
                    nc.vector.tensor_scalar(
                        yi[:], sse[:].bitcast(I32), 1, -1,
                        op0=LSR, op1=XOR)
                    nc.vector.tensor_scalar_add(yi[:], yi[:], MAGIC_P1)
                    y = yi[:].bitcast(F32)
                    na = workp.tile([128, H], F32, name="na", tag="tiny",
                                    bufs=12)
                    for _ in range(2):
                        nc.vector.tensor_mul(na[:], sse[:], y)
                        nc.vector.tensor_mul(na[:], na[:], y)
                        nc.vector.tensor_scalar(
                            na[:], na[:], -0.5, 1.5, op0=MULT, op1=ADD)
                        nc.vector.tensor_mul(y, y, na[:])
                    for h in range(H):
                        nc.vector.tensor_scalar_mul(
                            qbf[:, h, :], qbf[:, h, :],
                            yi[:, h:h + 1].bitcast(F32))
                    return qbf

                def qpipe_trans(qbf, bi, qT):
                    """transpose a scaled q tile into qT[:, :, bi-block]."""
                    for t3c in range(3):
                        # "acc" tag: av/sm are idle during the prologue
                        # (where ALL transposes now run), while the "proj"
                        # slots stay dedicated to the psq chains.
                        tps = psp.tile([128, 512], BF16, name="tps",
                                       tag="acc", bufs=2)
                        for cc in range(4):
                            h = t3c * 4 + cc
                            nc.tensor.transpose(
                                tps[:, cc * 128:(cc + 1) * 128],
                                qbf[:, h, :], ident_b)
                        nc.scalar.copy(
                            qT[:, t3c * 4:(t3c + 1) * 4,
                               bi * 128:(bi + 1) * 128],
                            tps[:].rearrange("p (a b) -> p a b", a=4))

                qTs = {}
                oTs = {}
                pend_tr = []                 # (qbf, block)
                proj_left = list(range(NBLK))

                def gi_of(b):
                    return 0 if b < GROUPS[1][0] else (
                        1 if b < GROUPS[2][0] else 2)

                def do_proj():
                    b = proj_left.pop(0)
                    gi = gi_of(b)
                    if b == GROUPS[gi][0]:
                        qTs[gi] = workp.tile([128, H, 512], BF16, name="qT",
                                             tag="t12", bufs=3)
                    pend_tr.append((qpipe_proj(b), b))

                def do_trans():
                    qbf, b = pend_tr.pop(0)
                    gi = gi_of(b)
                    qpipe_trans(qbf, b - GROUPS[gi][0], qTs[gi])

                # ------- prologue: PRO blocks, transposes trail by one ----
                do_proj()
                do_proj()
                for _ in range(PRO - 2):
                    do_trans()
                    do_proj()
                do_trans()
                do_trans()

                # ------- attention, software-pipelined one head deep ------
                def attn_scores(h, gw, qT):
                    """scores + exp for head h; rowsum partials on gpsimd/
                    DVE. Returns (pT, rsum)."""
                    pT = workp.tile([128, JB, 512], BF16, name="pT",
                                    tag="pT", bufs=2)
                    prt = workp.tile([128, 2, 512], BF16, name="prt",
                                     tag="prs", bufs=2)
                    for half in range(2):
                        sT = psp.tile([128, 2, 512], F32, name="sT",
                                      tag="sT", bufs=2)
                        for jj in range(2):
                            jb = half * 2 + jj
                            nc.tensor.matmul(
                                sT[:, jj, :gw],
                                lhsT=kT_sb[:, h, jb * 128:(jb + 1) * 128],
                                rhs=qT[:, h, :gw], start=True, stop=True)
                        nc.scalar.activation(
                            pT[:, half * 2:(half + 1) * 2, :gw],
                            sT[:, :, :gw], EXP)
                        nc.vector.tensor_add(
                            prt[:, half, :gw],
                            pT[:, half * 2, :gw],
                            pT[:, half * 2 + 1, :gw])
                    rsum = workp.tile([128, 512], BF16, name="rsum",
                                      tag="prs2", bufs=2)
                    nc.vector.tensor_add(
                        rsum[:, :gw], prt[:, 0, :gw], prt[:, 1, :gw])
                    return pT, rsum

                def attn_tail(h, gw, pT, rsum, oT):
                    """AV + single rowsum matmul + normalize for head h."""
                    av = psp.tile([128, 512], F32, name="av", tag="acc",
                                  bufs=2)
                    for jb in range(JB):
                        nc.tensor.matmul(
                            av[:, :gw],
                            lhsT=v_sb[:, jb, h * 128:(h + 1) * 128],
                            rhs=pT[:, jb, :gw],
                            start=(jb == 0), stop=(jb == JB - 1))
                    sm = psp.tile([128, 512], F32, name="sm", tag="acc",
                                  bufs=2)
                    nc.tensor.matmul(
                        sm[:, :gw], lhsT=ones_b[:], rhs=rsum[:, :gw],
                        start=True, stop=True)
                    rs = workp.tile([128, 512], F32, name="rs", tag="s2",
                                    bufs=4)
                    nc.vector.reciprocal_approx_fast(rs[:, :gw], sm[:, :gw])
                    nc.vector.tensor_mul(
                        oT[:, h, :gw], av[:, :gw], rs[:, :gw])

                def outproj_chunk(ib, bi, ec, oT):
                    sl = slice(ec * 512, (ec + 1) * 512)
                    po = psp.tile([128, 512], F32, name="po", tag="proj",
                                  bufs=2)
                    for hh in range(H):
                        nc.tensor.matmul(
                            po[:],
                            lhsT=oT[:, hh, bi * 128:(bi + 1) * 128],
                            rhs=wo_sb[:, hh, sl],
                            start=(hh == 0), stop=(hh == H - 1))
                    och = workp.tile([128, 512], F32, name="och", tag="s2",
                                     bufs=4)
                    nc.vector.tensor_copy(och[:], po[:])
                    nc.sync.dma_start(
                        out=out_d[ib * 128:(ib + 1) * 128, sl], in_=och[:])

                pend = None
                for gi, (g0b, gn) in enumerate(GROUPS):
                    gw = gn * 128
                    oTs[gi] = workp.tile([128, H, 512], BF16, name="oT",
                                         tag="oT", bufs=2)
                    op_left = []
                    if gi > 0:
                        pg0, pgn = GROUPS[gi - 1]
                        op_left = [(pg0 + bi, bi, ec)
                                   for bi in range(pgn) for ec in range(3)]
                    for h in range(H):
                        if h == 0 and pend is not None:
                            # finish prev group before its outproj fillers
                            attn_tail(*pend)
                            pend = None
                        cur = attn_scores(h, gw, qTs[gi])
                        # PE filler between scores(h) and tail(h-1): covers
                        # the exp+rowsum latency of head h.  Skipped at h==0
                        # so the first chunk never waits on the just-flushed
                        # last head of the previous group's oT.
                        if op_left and h > 0:
                            ib, bi, ec = op_left.pop(0)
                            outproj_chunk(ib, bi, ec, oTs[gi - 1])
                        elif gi == 0:
                            if h % 2 == 1 and proj_left:
                                do_proj()
                            elif pend_tr:
                                do_trans()
                        if pend is not None:
                            attn_tail(*pend)
                        pend = (h, gw, cur[0], cur[1], oTs[gi])
                    while op_left:
                        ib, bi, ec = op_left.pop(0)
                        outproj_chunk(ib, bi, ec, oTs[gi - 1])
                    while pend_tr:
                        do_trans()
                attn_tail(*pend)
                # epilogue: outproj of last group
                lg0, lgn = GROUPS[-1]
                for bi in range(lgn):
                    for ec in range(3):
                        outproj_chunk(lg0 + bi, bi, ec, oTs[len(GROUPS) - 1])

            if reps == 1:
                body()
            else:
                with tc.For_i(0, reps, 1):
                    body()
    nc.finalize()
    return nc


def kernel(x, context, wq, bq, wkv, bkv, wo, bo, q_norm_scale):
    x = np.asarray(x, dtype=np.float32)
    context = np.asarray(context, dtype=np.float32)
    bf = ml_dtypes.bfloat16

    if "nc" not in _cache:
        _cache["nc"] = _build()
    nc = _cache["nc"]

    scale_t = np.tile(np.asarray(q_norm_scale, np.float32), H)      # [D]
    wkv_f = np.asarray(wkv, np.float32)
    wk_b = (wkv_f[:, :D] * scale_t[None, :]).astype(bf)
    wv_b = np.ascontiguousarray(wkv_f[:, D:]).astype(bf)
    wq_b = np.asarray(wq, np.float32).astype(bf)
    wo_b = np.asarray(wo, np.float32).astype(bf)

    xp = np.zeros((B, CPB * RPC, D), np.float32)
    xp[:, :N] = x
    ctxT_b = [np.ascontiguousarray(context[b].T).astype(bf) for b in range(B)]

    in_maps = []
    for core in range(NCORES):
        b, q = divmod(core, CPB)
        xT = np.ascontiguousarray(xp[b, q * RPC:(q + 1) * RPC].T).astype(bf)
        in_maps.append({
            "xT": xT,
            "ctxT": ctxT_b[b],
            "wq": wq_b,
            "wkp": np.ascontiguousarray(wk_b[:, q * 384:(q + 1) * 384]),
            "wvp": np.ascontiguousarray(wv_b[:, q * 384:(q + 1) * 384]),
            "wo": wo_b,
        })

    res = bass_utils.run_bass_kernel_spmd(
        nc, in_maps, core_ids=list(range(NCORES)), trace=TRACE)
    _cache["last_results"] = res

    out = np.empty((B, N, D), np.float32)
    for b in range(B):
        cat = np.concatenate(
            [res.results[b * CPB + q]["out"] for q in range(CPB)], axis=0)
        out[b] = cat[:N]
    return out
